# revision 24
# baseline (speedup 1.0000x reference)
"""GCN (3-layer, PyG GCNConv-style) forward pass on 8 Trainium2 NeuronCores.

Architecture v5 (gather L1 + hot/cold split PE-scatter L2/L3, chunked AG):
  - Nodes are assigned to tiles by OUT-degree bands (band k -> tile k on
    every core; within a band, snake-dealt by IN-degree across cores), so
    tile index correlates with out-degree.  Tiles >= JCUT hold the
    lowest-out-degree nodes ("cold"), the rest are "hot".
  - Z_l = dis * (H @ Wl) computed per core and AllGathered in chunks:
    layer 1 in halves (a = tiles 0..3, b = 4..19); layers 2/3 in three
    chunks aligned with the hot/cold boundary (a = 0..3, b1 = 4..JCUT-1
    hot, b2 = JCUT..19 cold) so the hot scatter can begin as soon as b1
    lands while cold gathers wait for b2.  Layer-3 rows padded to 128
    (gather needs 256B rows).
  - Layer 1 aggregation: SWDGE dma_gather + fp8-selector matmuls (gather
    costs ~7 ns/row of GpSimd regardless of width -> it handles the widest
    layer).  Self-loops enter via an identity matmul on the local Z tile;
    gather calls use exact per-group counts on 4 SWDGE queues.
  - Layers 2/3, hot source tiles: PE-scatter.  Z_s is stationary, a 0/1
    adjacency slice S_s [128 src x 2560 dst] (fp8, self-loops included)
    streams as the moving operand (fetched two tiles per DMA on the
    Activation engine's HWDGE queue), accumulating a feature-major PSUM
    [d x 2560] in five 512-col bank chunks.
  - Layers 2/3, cold source tiles: edges dma_gathered (GpSimd idles during
    scatter otherwise) and folded into the same PSUM via per-dst-tile
    selector matmuls with the gathered block stationary.
  - The layer tail is pipelined per 512-col chunk: close chunk c (cold
    matmuls) -> feature-major post -> next-layer GEMM for its 4 node tiles
    -> AllGather chunk fires as soon as its tiles are stored.
  - Post-ops run feature-major (dis as a replicated row, bias per
    partition); the next GEMM consumes H_fm directly as lhsT; the final
    output is PE-transposed back to node-major.
"""

import sys

import numpy as np

sys.path.insert(0, "/opt/trn_rl_repo")

import ml_dtypes  # noqa: E402

import concourse.bass as bass  # noqa: E402
import concourse.bacc as bacc  # noqa: E402
import concourse.mybir as mybir  # noqa: E402
from concourse.bass_utils import run_bass_kernel_spmd  # noqa: E402
from concourse.library_config import mlp as _mlp_lib  # noqa: E402
from concourse.tile import TileContext  # noqa: E402
from concourse.tile_rust import add_dep_helper  # noqa: E402

BF16 = ml_dtypes.bfloat16
FP8 = ml_dtypes.float8_e4m3

# ----------------------------------------------------------------------------
# Problem configuration (hardcoded for nn_Encoder_17386027614431)
# ----------------------------------------------------------------------------
N_NODES = 20000
N_CORES = 8
T = 128
NT = 20                  # dst tiles per core
SHARD = NT * T           # 2560
NTAB = N_CORES * SHARD   # 20480 table rows
D0 = 256
DL = [256, 128, 64]      # per-layer output dims
HT_A = 4                 # tiles in AllGather chunk a
JCUT = 11                # tiles >= JCUT are cold: L2/L3 edges via gather
HALF_A = HT_A * T
HALF_B = SHARD - HALF_A
SA_TILES = N_CORES * HT_A          # 32 src tiles in chunk-a table
HOTB = JCUT - HT_A                 # hot b1 tiles per core (7)
NCOLD = NT - JCUT                  # cold tiles per core (9)
DPAD = [256, 128, 128]             # table row widths (L3 padded)
CCHUNK = 512                       # psum bank columns (f32)
NCH = SHARD // CCHUNK              # 5 feature-major column chunks
# AG chunk tile ranges per layer
AGCH = [
    [(0, HT_A), (HT_A, NT)],
    [(0, HT_A), (HT_A, JCUT), (JCUT, NT)],
    [(0, HT_A), (HT_A, JCUT), (JCUT, NT)],
]


def _ru16(x):
    return (int(x) + 15) // 16 * 16


def _offsets(cnt2d):
    flat = [c for row in cnt2d for c in row]
    nbl = [(c + T - 1) // T for c in flat]
    boff, ioff = [], []
    ob = oi = 0
    for c, nb in zip(flat, nbl):
        boff.append(ob)
        ioff.append(oi)
        ob += nb
        oi += c // 16
    return nbl, boff, ioff, ob, oi


def _build_nc(CNT, CNT2, apply_b1, apply_b2, apply_b3):
    f32 = mybir.dt.float32
    bf16 = mybir.dt.bfloat16
    fp8 = mybir.dt.float8e4
    i16 = mybir.dt.int16
    mult = mybir.AluOpType.mult
    add = mybir.AluOpType.add
    relu = mybir.ActivationFunctionType.Relu
    fcopy = mybir.ActivationFunctionType.Copy

    nbl_f, boff_f, ioff_f, totblk, idxcols = _offsets(CNT)
    nbl = [nbl_f[:NT], nbl_f[NT:]]
    boff = [boff_f[:NT], boff_f[NT:]]
    ioff = [ioff_f[:NT], ioff_f[NT:]]
    nbl2, boff2, ioff2, totblk2, idxcols2 = _offsets([CNT2])
    maxnb = max(max(nbl[0]), max(nbl[1]), max(nbl2))

    nc = bacc.Bacc("TRN2", num_devices=N_CORES, num_swdge_queues=4)

    # ---- kernel I/O ----
    xt = nc.dram_tensor("xt", [D0, SHARD], bf16, kind="ExternalInput")
    w1 = nc.dram_tensor("w1", [D0, DL[0]], bf16, kind="ExternalInput")
    w2 = nc.dram_tensor("w2", [DL[0], DL[1]], bf16, kind="ExternalInput")
    w3 = nc.dram_tensor("w3", [DL[1], DL[2]], bf16, kind="ExternalInput")
    brep1 = nc.dram_tensor("brep1", [T, DL[0]], f32, kind="ExternalInput")
    b2col = nc.dram_tensor("b2col", [T, 1], f32, kind="ExternalInput")
    b3col = nc.dram_tensor("b3col", [T, 1], f32, kind="ExternalInput")
    dis = nc.dram_tensor("dis", [T, NT], f32, kind="ExternalInput")
    dis2 = nc.dram_tensor("dis2", [T, NT], f32, kind="ExternalInput")
    disrow = nc.dram_tensor("disrow", [T, SHARD], f32, kind="ExternalInput")
    disrow2 = nc.dram_tensor("disrow2", [T, SHARD], f32,
                             kind="ExternalInput")
    identb = nc.dram_tensor("identb", [T, T], bf16, kind="ExternalInput")
    identf = nc.dram_tensor("identf", [T, T], f32, kind="ExternalInput")
    idx = nc.dram_tensor("idx", [T, idxcols], i16, kind="ExternalInput")
    sel = nc.dram_tensor("sel", [T, totblk * T], fp8, kind="ExternalInput")
    idx2 = nc.dram_tensor("idx2", [T, idxcols2], i16, kind="ExternalInput")
    sel2 = nc.dram_tensor("sel2", [T, totblk2 * T], fp8,
                          kind="ExternalInput")
    smat = nc.dram_tensor("smat", [NTAB, SHARD], fp8, kind="ExternalInput")
    out = nc.dram_tensor("out", [SHARD, DL[2]], f32, kind="ExternalOutput")

    # ---- internal DRAM for collectives (per layer, per AG chunk) ----
    agin, agout = [], []
    for l in range(3):
        ai, ao = [], []
        for k, (j0, j1) in enumerate(AGCH[l]):
            rows = (j1 - j0) * T
            ai.append(nc.dram_tensor(f"agin{l}_{k}", [rows, DPAD[l]], bf16))
            ao.append(nc.dram_tensor(
                f"agout{l}_{k}", [N_CORES * rows, DPAD[l]], bf16,
                addr_space="Shared"))
        agin.append(ai)
        agout.append(ao)
    rg = [list(range(N_CORES))]

    with TileContext(nc) as tc:
        nc.gpsimd.load_library(_mlp_lib)

        with (
            tc.tile_pool(name="const", bufs=1) as cpool,
            tc.tile_pool(name="sb", bufs=4) as sbpool,        # S stream
            tc.tile_pool(name="zsb", bufs=2) as zspool,       # Z stationary
            tc.tile_pool(name="selp", bufs=3) as selpool,
            tc.tile_pool(name="hp", bufs=2) as hpool,
            tc.tile_pool(name="htp", bufs=3) as htpool,
            tc.tile_pool(name="tmp", bufs=3) as tpool,
            tc.tile_pool(name="zbp", bufs=3) as zbpool,
            tc.tile_pool(name="ps_z", bufs=1, space="PSUM") as ps_z,
            tc.tile_pool(name="ps_agg", bufs=1, space="PSUM") as ps_agg,
            tc.tile_pool(name="ps_t", bufs=1, space="PSUM") as ps_t,
            tc.tile_pool(name="ps_fm", bufs=1, space="PSUM") as ps_fm,
        ):
            # ---- constants (xt/w1/dis first so Z1 starts immediately) ----
            def load_const(dram_h, shape, dtype):
                t = cpool.tile(shape, dtype, tag=f"c_{dram_h.name}")
                nc.sync.dma_start(out=t[:, :], in_=dram_h.ap())
                return t

            def load_const_chunked(dram_h, inner, dtype):
                cs = dram_h.shape[0] // T
                t = cpool.tile([T, cs * inner], dtype, tag=f"c_{dram_h.name}")
                nc.sync.dma_start(
                    out=t.rearrange("p (c n) -> p c n", c=cs),
                    in_=dram_h.ap().rearrange("(c p) n -> p c n", p=T),
                )
                return t

            xt_sb = load_const_chunked(xt, SHARD, bf16)
            w1_sb = load_const_chunked(w1, DL[0], bf16)
            dis_sb = load_const(dis, [T, NT], f32)
            dis2_sb = load_const(dis2, [T, NT], f32)
            identb_sb = load_const(identb, [T, T], bf16)
            idx_sb = load_const(idx, [T, idxcols], i16)
            idx2_sb = load_const(idx2, [T, idxcols2], i16)
            w2_sb = load_const_chunked(w2, DL[1], bf16)
            w3_sb = load_const(w3, [DL[1], DL[2]], bf16)
            brep1_sb = load_const(brep1, [T, DL[0]], f32)
            b2_sb = load_const(b2col, [T, 1], f32)
            b3_sb = load_const(b3col, [T, 1], f32)
            disrow_sb = load_const(disrow, [T, SHARD], f32)
            disrow2_sb = load_const(disrow2, [T, SHARD], f32)
            identf_sb = load_const(identf, [T, T], f32)

            # persistent buffers
            gbuf = [cpool.tile([T, maxnb * DL[0]], bf16, tag=f"g{i}",
                               name=f"gbuf{i}") for i in range(3)]
            for g in gbuf:
                nc.vector.memset(g[:, :], 0.0)
            zb1 = [cpool.tile([T, DL[0]], bf16, tag=f"zb1_{j}",
                              name=f"zb1_{j}") for j in range(NT)]
            acc = cpool.tile([T, NT * DL[0]], f32, tag="acc")
            h2fm = cpool.tile([T, SHARD], bf16, tag="h2fm")
            outfm = cpool.tile([T, SHARD], f32, tag="outfm")
            fm = [ps_fm.tile([T, CCHUNK], f32, tag=f"fm{c}",
                             name=f"fm{c}") for c in range(NCH)]
            nb2max = max(nbl2)
            cbuf = [cpool.tile([T, nb2max * DPAD[1]], bf16, tag=f"cb{j}",
                               name=f"cbuf{j}") for j in range(NT)]
            for cb in cbuf:
                nc.vector.memset(cb[:, :], 0.0)

            agin_v = [[agin[l][k].ap().rearrange("(n p) d -> p n d", p=T)
                       for k in range(len(AGCH[l]))] for l in range(3)]
            agout_v = [[agout[l][k].ap().rearrange("(n p) d -> p n d", p=T)
                        for k in range(len(AGCH[l]))] for l in range(3)]
            smat_v = smat.ap().rearrange("(s p) d -> p s d", p=T)
            out_v = out.ap().rearrange("(n p) d -> p n d", p=T)

            ag_insts = [[None] * len(AGCH[l]) for l in range(3)]
            agin_dmas = [[[] for _ in AGCH[l]] for l in range(3)]

            def z_store(l, j, zb):
                for k, (j0, j1) in enumerate(AGCH[l]):
                    if j0 <= j < j1:
                        break
                d = nc.sync.dma_start(
                    out=agin_v[l][k][:, j - j0, :], in_=zb[:, :])
                agin_dmas[l][k].append(d)

            def issue_ag(l, k):
                cc = nc.gpsimd.collective_compute(
                    "AllGather",
                    mybir.AluOpType.bypass,
                    replica_groups=rg,
                    ins=[agin[l][k].ap().opt()],
                    outs=[agout[l][k].ap().opt()],
                )
                for d in agin_dmas[l][k]:
                    add_dep_helper(cc.ins, d.ins, reason=f"ag{l}.{k}")
                ag_insts[l][k] = cc

            # ================= Layer 1: Z1 = (dis*x) @ W1 ==================
            # two tiles share one [T,512] psum bank; copies on scalar engine
            for jp in range(NT // 2):
                zp = ps_z.tile([T, 2 * DL[0]], f32, tag="zp")
                for half in range(2):
                    j = 2 * jp + half
                    o = half * DL[0]
                    for c in range(2):
                        nc.tensor.matmul(
                            zp[:, o:o + DL[0]],
                            xt_sb[:, c * SHARD + j * T:
                                  c * SHARD + (j + 1) * T],
                            w1_sb[:, c * DL[0]:(c + 1) * DL[0]],
                            start=(c == 0), stop=(c == 1),
                        )
                for half in range(2):
                    j = 2 * jp + half
                    nc.scalar.activation(
                        zb1[j][:, :], zp[:, half * DL[0]:(half + 1) * DL[0]],
                        fcopy)
                    z_store(0, j, zb1[j])
                if 2 * jp + 1 == HT_A - 1:
                    issue_ag(0, 0)
            issue_ag(0, 1)

            # ---- L1 gather helper ----
            gq = [0]

            def gather_group(h, j, gslot):
                cnt = CNT[h][j]
                nb = nbl[h][j]
                gt3 = gbuf[gslot][:, :nb * DL[0]].rearrange(
                    "p (n d) -> p n d", d=DL[0])
                g = nc.gpsimd.dma_gather(
                    gt3,
                    agout[0][h].ap(),
                    idx_sb[:, ioff[h][j]:ioff[h][j] + cnt // 16],
                    cnt, cnt, DL[0],
                    single_packet=False,
                    queue_num=gq[0] % 4,
                )
                gq[0] += 1
                add_dep_helper(g.ins, ag_insts[0][h].ins, reason="g ag")
                st = selpool.tile([T, maxnb * T], fp8, tag="sel")
                nc.sync.dma_start(
                    out=st[:, :nb * T],
                    in_=sel[:, boff[h][j] * T:(boff[h][j] + nb) * T])
                return gt3, st, nb

            # ---- cold gathers / matmuls for L2/L3 ----
            def cold_gathers(l):
                for j in range(NT):
                    cnt = CNT2[j]
                    gt3 = cbuf[j][:, :nbl2[j] * DPAD[l]].rearrange(
                        "p (n d) -> p n d", d=DPAD[l])
                    g = nc.gpsimd.dma_gather(
                        gt3,
                        agout[l][2].ap(),
                        idx2_sb[:, ioff2[j]:ioff2[j] + cnt // 16],
                        cnt, cnt, DPAD[l],
                        single_packet=False,
                        queue_num=gq[0] % 4,
                    )
                    gq[0] += 1
                    add_dep_helper(g.ins, ag_insts[l][2].ins, reason="cg ag")

            def cold_matmuls_chunk(l, c):
                """Fold cold edges of dst tiles 4c..4c+3 into fm[c]; the last
                one closes the accumulation group."""
                d_el = DL[l]
                for j in range(4 * c, 4 * c + 4):
                    nb = nbl2[j]
                    gt3 = cbuf[j][:, :nb * DPAD[l]].rearrange(
                        "p (n d) -> p n d", d=DPAD[l])
                    st = selpool.tile([T, maxnb * T], fp8, tag="sel")
                    nc.sync.dma_start(
                        out=st[:, :nb * T],
                        in_=sel2[:, boff2[j] * T:(boff2[j] + nb) * T])
                    r = (j % 4) * T
                    for b in range(nb):
                        nc.tensor.matmul(
                            fm[c][:d_el, r:r + T],
                            gt3[:, b, :d_el],
                            st[:, b * T:(b + 1) * T],
                            start=False,
                            stop=(j % 4 == 3 and b == nb - 1),
                            skip_group_check=True)

            # ---- hot scatter: chunk-a tiles then per-core b1 stripes; S
            # fetched two tiles per DMA on the Activation HWDGE queue. ----
            fetch_plan = []           # (smat_row0, ntiles, ag chunk)
            for g2 in range(SA_TILES // 2):
                fetch_plan.append((2 * g2, 2, 0))
            for core in range(N_CORES):
                base = SA_TILES + core * HOTB
                k = 0
                while k < HOTB:
                    n = min(2, HOTB - k)
                    fetch_plan.append((base + k, n, 1))
                    k += n
            hot_steps = []            # (fetch_idx, k_in_fetch)
            for fi, (r0, n, ch) in enumerate(fetch_plan):
                for k in range(n):
                    hot_steps.append((fi, k))
            n_hot = len(hot_steps)

            scat = {"pos": 0, "zsb": None, "stile": None}

            def scatter_steps(l, n, limit):
                d_el = DL[l]
                dp = DPAD[l]
                while n > 0 and scat["pos"] < limit:
                    pos = scat["pos"]
                    fi, k = hot_steps[pos]
                    r0, fn, ch = fetch_plan[fi]
                    if pos < SA_TILES:
                        if pos % 8 == 0:
                            zsb = zspool.tile([T, 8 * dp], bf16,
                                              tag=f"zsa{l}")
                            d = nc.sync.dma_start(
                                out=zsb.rearrange("p (n d) -> p n d", d=dp),
                                in_=agout_v[l][0][:, pos:pos + 8, :])
                            add_dep_helper(d.ins, ag_insts[l][0].ins,
                                           reason="zs ag")
                            scat["zsb"] = zsb
                        zk = pos % 8
                    else:
                        p = pos - SA_TILES
                        if p % HOTB == 0:
                            core = p // HOTB
                            zsb = zspool.tile([T, HOTB * dp], bf16,
                                              tag=f"zsb{l}")
                            d = nc.sync.dma_start(
                                out=zsb.rearrange("p (n d) -> p n d", d=dp),
                                in_=agout_v[l][1][:, core * HOTB:
                                                  (core + 1) * HOTB, :])
                            add_dep_helper(d.ins, ag_insts[l][1].ins,
                                           reason="zs ag")
                            scat["zsb"] = zsb
                        zk = p % HOTB
                    if k == 0:
                        stile = sbpool.tile([T, 2 * SHARD], fp8, tag="sm")
                        nc.scalar.dma_start(
                            out=stile[:, :fn * SHARD].rearrange(
                                "p (n d) -> p n d", d=SHARD),
                            in_=smat_v[:, r0:r0 + fn, :])
                        scat["stile"] = stile
                    stile = scat["stile"]
                    zsb = scat["zsb"]
                    for c in range(NCH):
                        nc.tensor.matmul(
                            fm[c][:d_el, :],
                            zsb[:, zk * dp:zk * dp + d_el],
                            stile[:, k * SHARD + c * CCHUNK:
                                  k * SHARD + (c + 1) * CCHUNK],
                            start=(pos == 0), stop=False,
                            skip_group_check=True)
                    scat["pos"] = pos + 1
                    n -= 1

            # ================= L1 phase A (src chunk a) ====================
            for j in range(NT):
                gt3, st, nb = gather_group(0, j, j % 3)
                ps = ps_agg.tile([T, DL[0]], f32, tag="agg")
                nc.tensor.matmul(ps[:, :], identb_sb[:, :], zb1[j][:, :],
                                 start=True, stop=False)
                for b in range(nb):
                    nc.tensor.matmul(
                        ps[:, :], st[:, b * T:(b + 1) * T], gt3[:, b, :],
                        start=False, stop=(b == nb - 1))
                nc.scalar.activation(
                    acc[:, j * DL[0]:(j + 1) * DL[0]], ps[:, :], fcopy)

            # ================= L1 phase B + post + Z2 + AG2 ================
            for j in range(NT):
                gt3, st, nb = gather_group(1, j, j % 3)
                ps = ps_agg.tile([T, DL[0]], f32, tag="agg")
                for b in range(nb):
                    nc.tensor.matmul(
                        ps[:, :], st[:, b * T:(b + 1) * T], gt3[:, b, :],
                        start=(b == 0), stop=(b == nb - 1))
                u = tpool.tile([T, DL[0]], f32, tag="post")
                nc.vector.tensor_tensor(
                    u[:, :], ps[:, :], acc[:, j * DL[0]:(j + 1) * DL[0]], add)
                h1 = hpool.tile([T, DL[0]], bf16, tag="h1")
                if apply_b1:
                    # generic path: h1 = relu(dis*u + b1); zb2 prescaled later
                    t1 = tpool.tile([T, DL[0]], f32, tag="post")
                    nc.vector.tensor_scalar(
                        t1[:, :], u[:, :], dis_sb[:, j:j + 1], None, mult)
                    t2 = tpool.tile([T, DL[0]], f32, tag="post")
                    nc.vector.tensor_tensor(t2[:, :], t1[:, :],
                                            brep1_sb[:, :], add)
                    nc.scalar.activation(h1[:, :], t2[:, :], relu)
                else:
                    # h1 = dis*relu(dis*u) = relu(dis^2*u): Z2 comes out
                    # pre-scaled for the next aggregation
                    nc.scalar.activation(h1[:, :], u[:, :], relu,
                                         scale=dis2_sb[:, j:j + 1])
                zp2 = ps_z.tile([T, 2 * DL[0]], f32, tag="zp")
                for c in range(2):
                    tp = ps_t.tile([T, T], bf16, tag="tp")
                    nc.tensor.matmul(tp[:, :], h1[:, c * T:(c + 1) * T],
                                     identb_sb[:, :], is_transpose=True)
                    htc = htpool.tile([T, T], bf16, tag="ht")
                    nc.scalar.activation(htc[:, :], tp[:, :], fcopy)
                    nc.tensor.matmul(
                        zp2[:, :DL[1]], htc[:, :],
                        w2_sb[:, c * DL[1]:(c + 1) * DL[1]],
                        start=(c == 0), stop=(c == 1))
                zb2 = zbpool.tile([T, DL[1]], bf16, tag="zb2")
                if apply_b1:
                    nc.vector.tensor_scalar(
                        zb2[:, :], zp2[:, :DL[1]], dis_sb[:, j:j + 1],
                        None, mult)
                else:
                    nc.scalar.activation(zb2[:, :], zp2[:, :DL[1]], fcopy)
                z_store(1, j, zb2)
                if j == HT_A - 1:
                    issue_ag(1, 0)
                if j == JCUT - 1:
                    issue_ag(1, 1)
                if j == NT - 1:
                    issue_ag(1, 2)
                if j >= HT_A:
                    lim = SA_TILES if j < JCUT else n_hot
                    scatter_steps(1, 3, lim)

            # ---- layer tails: finish aggregation, pipeline per chunk ----
            def layer_tail(l):
                last = l == 2
                cold_gathers(l)
                scatter_steps(l, n_hot, n_hot)
                for c in range(NCH):
                    cold_matmuls_chunk(l, c)
                    d_el = DL[l]
                    t = tpool.tile([T, CCHUNK], f32, tag="fmpost")
                    if not last:
                        drow = disrow_sb if apply_b2 else disrow2_sb
                        nc.vector.tensor_tensor(
                            t[:d_el, :], fm[c][:d_el, :],
                            drow[:d_el, c * CCHUNK:(c + 1) * CCHUNK], mult)
                        nc.scalar.activation(
                            h2fm[:, c * CCHUNK:(c + 1) * CCHUNK],
                            t[:d_el, :], relu, bias=b2_sb[:, :])
                        for j in range(4 * c, 4 * c + 4):
                            zp3 = ps_z.tile([T, 2 * DL[0]], f32, tag="zp")
                            nc.tensor.matmul(
                                zp3[:, :DL[2]], h2fm[:, j * T:(j + 1) * T],
                                w3_sb[:, :], start=True, stop=True)
                            zb3 = zbpool.tile([T, DPAD[2]], bf16, tag="zb3")
                            if apply_b2:
                                nc.vector.tensor_scalar(
                                    zb3[:, :DL[2]], zp3[:, :DL[2]],
                                    dis_sb[:, j:j + 1], None, mult)
                            else:
                                nc.scalar.activation(
                                    zb3[:, :DL[2]], zp3[:, :DL[2]], fcopy)
                            z_store(2, j, zb3)
                            if j == HT_A - 1:
                                issue_ag(2, 0)
                            if j == JCUT - 1:
                                issue_ag(2, 1)
                            if j == NT - 1:
                                issue_ag(2, 2)
                    else:
                        nc.vector.tensor_tensor(
                            t[:d_el, :], fm[c][:d_el, :],
                            disrow_sb[:d_el, c * CCHUNK:(c + 1) * CCHUNK],
                            mult)
                        if apply_b3:
                            nc.scalar.activation(
                                outfm[:DL[2], c * CCHUNK:(c + 1) * CCHUNK],
                                t[:DL[2], :], fcopy, bias=b3_sb[:DL[2], :])
                        else:
                            nc.scalar.activation(
                                outfm[:DL[2], c * CCHUNK:(c + 1) * CCHUNK],
                                t[:DL[2], :], fcopy)
                        for j in range(4 * c, 4 * c + 4):
                            tpf = ps_z.tile([T, 2 * DL[0]], f32, tag="zp")
                            nc.tensor.matmul(
                                tpf[:, :DL[2]],
                                outfm[:DL[2], j * T:(j + 1) * T],
                                identf_sb[:DL[2], :DL[2]],
                                is_transpose=True)
                            ot = htpool.tile([T, DL[2]], f32, tag="ot")
                            nc.scalar.activation(ot[:, :], tpf[:, :DL[2]],
                                                 fcopy)
                            nc.sync.dma_start(out=out_v[:, j, :],
                                              in_=ot[:, :])

            layer_tail(1)
            scat["pos"] = 0
            layer_tail(2)

    nc.compile()
    return nc


# ----------------------------------------------------------------------------
# Host-side preprocessing
# ----------------------------------------------------------------------------
def _band_node_order(outdeg, indeg):
    by_out = np.argsort(-outdeg, kind="stable")
    node_order = -np.ones(NTAB, np.int64)
    new_pos = np.zeros(N_NODES, np.int64)
    band_sz = N_CORES * T
    for k in range(NT):
        band = by_out[k * band_sz:(k + 1) * band_sz]
        band = band[np.argsort(-indeg[band], kind="stable")]
        fill = np.zeros(N_CORES, np.int64)
        b = 0
        direction = 1
        for node in band:
            pos = b * SHARD + k * T + fill[b]
            node_order[pos] = node
            new_pos[node] = pos
            fill[b] += 1
            b += direction
            if b == N_CORES:
                b = N_CORES - 1
                direction = -1
            elif b < 0:
                b = 0
                direction = 1
    return node_order, new_pos


def _group_pack(core_s, grp, ngrp, row_s, slot_s, CNT_flat, ioff_flat,
                boff_flat):
    grp_start = np.zeros(N_CORES * ngrp + 1, np.int64)
    np.add.at(grp_start, core_s * ngrp + grp + 1, 1)
    grp_start = np.cumsum(grp_start)
    rank = np.arange(len(grp)) - grp_start[core_s * ngrp + grp]
    cnt_np = np.array(CNT_flat)
    ioff_np = np.array(ioff_flat)
    boff_np = np.array(boff_flat)
    epos = ioff_np[grp] * 16 + rank
    blk = boff_np[grp] + rank // T
    lane = rank % T
    idxcols = int(ioff_np[-1] + cnt_np[-1] // 16)
    totblk = int(boff_np[-1] + (cnt_np[-1] + T - 1) // T)
    idx_cores, sel_cores = [], []
    for c in range(N_CORES):
        m = core_s == c
        flat = np.zeros(idxcols * 16, np.int16)
        flat[epos[m]] = row_s[m].astype(np.int16)
        wrapped = np.tile(flat.reshape(idxcols, 16).T, (8, 1))
        idx_cores.append(np.ascontiguousarray(wrapped.astype(np.int16)))
        selc = np.zeros((totblk, T, T), np.uint8)
        selc[blk[m], lane[m], slot_s[m]] = 1
        sel_cores.append(np.ascontiguousarray(
            selc.transpose(1, 0, 2).reshape(T, totblk * T)).astype(FP8))
    return idx_cores, sel_cores


def _preprocess(edge_index):
    src = np.asarray(edge_index[0], dtype=np.int64)
    dst = np.asarray(edge_index[1], dtype=np.int64)
    indeg = np.bincount(dst, minlength=N_NODES).astype(np.float64) + 1.0
    outdeg = np.bincount(src, minlength=N_NODES).astype(np.float64)
    dis_full = 1.0 / np.sqrt(indeg)

    node_order, new_pos = _band_node_order(outdeg, indeg)

    spos = new_pos[src]
    dpos = new_pos[dst]
    core = dpos // SHARD
    tile = (dpos % SHARD) // T
    slot = dpos % T
    shalf = ((spos % SHARD) >= HALF_A).astype(np.int64)
    srow_half = ((spos // SHARD) * np.where(shalf == 0, HALF_A, HALF_B)
                 + (spos % SHARD) - shalf * HALF_A)

    counts = np.zeros((N_CORES, 2, NT), np.int64)
    np.add.at(counts, (core, shalf, tile), 1)
    CNT = [[max(16, _ru16(counts[:, h, j].max())) for j in range(NT)]
           for h in range(2)]
    CNT_flat = [c for row in CNT for c in row]
    _, boff_f, ioff_f, _, _ = _offsets(CNT)

    order = np.lexsort((slot, tile, shalf, core))
    g1 = shalf[order] * NT + tile[order]
    idx_cores, sel_cores = _group_pack(
        core[order], g1, 2 * NT, srow_half[order], slot[order],
        CNT_flat, ioff_f, boff_f)

    # ---- cold edges (src tile >= JCUT -> AG chunk 2) for L2/L3 ----
    loop_pos = new_pos[node_order[node_order >= 0]]
    s_all = np.concatenate([spos, loop_pos])
    d_all = np.concatenate([dpos, loop_pos])
    stile_all = (s_all % SHARD) // T
    cold_m = stile_all >= JCUT
    sc = s_all[cold_m]
    dc = d_all[cold_m]
    ccore = dc // SHARD
    ctile = (dc % SHARD) // T
    cslot = dc % T
    crow = (sc // SHARD) * (NCOLD * T) + (sc % SHARD) - JCUT * T
    counts2 = np.zeros((N_CORES, NT), np.int64)
    np.add.at(counts2, (ccore, ctile), 1)
    CNT2 = [max(16, _ru16(counts2[:, j].max())) for j in range(NT)]
    _, boff2_f, ioff2_f, _, _ = _offsets([CNT2])
    order2 = np.lexsort((cslot, ctile, ccore))
    idx2_cores, sel2_cores = _group_pack(
        ccore[order2], ctile[order2], NT, crow[order2], cslot[order2],
        CNT2, ioff2_f, boff2_f)

    # ---- S matrices: hot srcs only, rows in [a | b1-hot] order ----
    hot_m = ~cold_m
    sh = s_all[hot_m]
    dh = d_all[hot_m]
    sh_tile = (sh % SHARD) // T
    sh_core = sh // SHARD
    sh_off = sh % T
    in_a = sh_tile < HT_A
    srow_glob = np.where(
        in_a,
        sh_core * HALF_A + sh_tile * T + sh_off,
        N_CORES * HALF_A + sh_core * (HOTB * T)
        + (sh_tile - HT_A) * T + sh_off)
    dcore_h = dh // SHARD
    dloc_h = dh % SHARD
    smat_cores = []
    for c in range(N_CORES):
        m = dcore_h == c
        S = np.zeros((NTAB, SHARD), np.uint8)
        np.add.at(S, (srow_glob[m], dloc_h[m]), 1)
        smat_cores.append(S.astype(FP8))

    dis_cores, disrow_cores = [], []
    for c in range(N_CORES):
        slots = node_order[c * SHARD:(c + 1) * SHARD]
        dis_c = np.where(slots >= 0, dis_full[np.maximum(slots, 0)], 0.0)
        dis_cores.append(np.ascontiguousarray(
            dis_c.reshape(NT, T).T).astype(np.float32))
        disrow_cores.append(np.ascontiguousarray(
            np.tile(dis_c[None, :], (T, 1))).astype(np.float32))

    return (idx_cores, sel_cores, idx2_cores, sel2_cores, dis_cores,
            disrow_cores, smat_cores, CNT, CNT2, node_order)


def _make_in_maps(x, W1, b1, W2, b2, W3, b3, edge_index):
    (idx_cores, sel_cores, idx2_cores, sel2_cores, dis_cores, disrow_cores,
     smat_cores, CNT, CNT2, node_order) = _preprocess(edge_index)

    x = np.asarray(x, np.float32)
    w1b = np.asarray(W1, np.float32).astype(BF16)
    w2b = np.asarray(W2, np.float32).astype(BF16)
    w3b = np.asarray(W3, np.float32).astype(BF16)
    b1f = np.asarray(b1, np.float32)
    b2f = np.asarray(b2, np.float32)
    b3f = np.asarray(b3, np.float32)
    brep1 = np.tile(b1f, (T, 1))
    b2col = np.zeros((T, 1), np.float32)
    b2col[:DL[1], 0] = b2f
    b3col = np.zeros((T, 1), np.float32)
    b3col[:DL[2], 0] = b3f
    identb = np.eye(T, dtype=BF16)
    identf = np.eye(T, dtype=np.float32)
    apply_b1 = bool(np.any(b1f))
    apply_b2 = bool(np.any(b2f))
    apply_b3 = bool(np.any(b3f))

    in_maps = []
    for c in range(N_CORES):
        slots = node_order[c * SHARD:(c + 1) * SHARD]
        xs = np.where((slots >= 0)[:, None], x[np.maximum(slots, 0)], 0.0)
        xs = xs * disrow_cores[c][0][:, None]
        in_maps.append({
            "xt": np.ascontiguousarray(xs.T.astype(np.float32)).astype(BF16),
            "w1": w1b, "w2": w2b, "w3": w3b,
            "brep1": brep1, "b2col": b2col, "b3col": b3col,
            "dis": dis_cores[c], "dis2": dis_cores[c] ** 2,
            "disrow": disrow_cores[c], "disrow2": disrow_cores[c] ** 2,
            "identb": identb, "identf": identf,
            "idx": idx_cores[c], "sel": sel_cores[c],
            "idx2": idx2_cores[c], "sel2": sel2_cores[c],
            "smat": smat_cores[c],
        })
    return in_maps, CNT, CNT2, node_order, apply_b1, apply_b2, apply_b3


_NC_CACHE = {}


def kernel_with_results(x, W1, b1, W2, b2, W3, b3, edge_index, trace=False):
    (in_maps, CNT, CNT2, node_order, apply_b1, apply_b2,
     apply_b3) = _make_in_maps(x, W1, b1, W2, b2, W3, b3, edge_index)
    key = (tuple(CNT[0]), tuple(CNT[1]), tuple(CNT2), apply_b1, apply_b2,
           apply_b3)
    if key not in _NC_CACHE:
        _NC_CACHE[key] = _build_nc(CNT, CNT2, apply_b1, apply_b2, apply_b3)
    nc = _NC_CACHE[key]
    res = run_bass_kernel_spmd(
        nc, in_maps, core_ids=list(range(N_CORES)), trace=trace)
    rows = np.concatenate(
        [np.asarray(res.results[c]["out"]) for c in range(N_CORES)], axis=0)
    full = np.zeros((N_NODES, rows.shape[1]), np.float32)
    real = node_order >= 0
    full[node_order[real]] = rows[real]
    return full, res


def kernel(x, W1, b1, W2, b2, W3, b3, edge_index):
    full, _ = kernel_with_results(x, W1, b1, W2, b2, W3, b3, edge_index)
    return full


# revision 25
# speedup vs baseline: 1.0072x; 1.0072x over previous
"""GCN (3-layer, PyG GCNConv-style) forward pass on 8 Trainium2 NeuronCores.

Architecture v5 (gather L1 + hot/cold split PE-scatter L2/L3, chunked AG):
  - Nodes are assigned to tiles by OUT-degree bands (band k -> tile k on
    every core; within a band, snake-dealt by IN-degree across cores), so
    tile index correlates with out-degree.  Tiles >= JCUT hold the
    lowest-out-degree nodes ("cold"), the rest are "hot".
  - Z_l = dis * (H @ Wl) computed per core and AllGathered in chunks:
    layer 1 in halves (a = tiles 0..3, b = 4..19); layers 2/3 in three
    chunks aligned with the hot/cold boundary (a = 0..3, b1 = 4..JCUT-1
    hot, b2 = JCUT..19 cold) so the hot scatter can begin as soon as b1
    lands while cold gathers wait for b2.  Layer-3 rows padded to 128
    (gather needs 256B rows).
  - Layer 1 aggregation: SWDGE dma_gather + fp8-selector matmuls (gather
    costs ~7 ns/row of GpSimd regardless of width -> it handles the widest
    layer).  Self-loops enter via an identity matmul on the local Z tile;
    gather calls use exact per-group counts on 4 SWDGE queues.
  - Layers 2/3, hot source tiles: PE-scatter.  Z_s is stationary, a 0/1
    adjacency slice S_s [128 src x 2560 dst] (fp8, self-loops included)
    streams as the moving operand (fetched two tiles per DMA on the
    Activation engine's HWDGE queue), accumulating a feature-major PSUM
    [d x 2560] in five 512-col bank chunks.
  - Layers 2/3, cold source tiles: edges dma_gathered (GpSimd idles during
    scatter otherwise) and folded into the same PSUM via per-dst-tile
    selector matmuls with the gathered block stationary.
  - The layer tail is pipelined per 512-col chunk: close chunk c (cold
    matmuls) -> feature-major post -> next-layer GEMM for its 4 node tiles
    -> AllGather chunk fires as soon as its tiles are stored.
  - Post-ops run feature-major (dis as a replicated row, bias per
    partition); the next GEMM consumes H_fm directly as lhsT; the final
    output is PE-transposed back to node-major.
"""

import sys

import numpy as np

sys.path.insert(0, "/opt/trn_rl_repo")

import ml_dtypes  # noqa: E402

import concourse.bass as bass  # noqa: E402
import concourse.bacc as bacc  # noqa: E402
import concourse.mybir as mybir  # noqa: E402
from concourse.bass_utils import run_bass_kernel_spmd  # noqa: E402
from concourse.library_config import mlp as _mlp_lib  # noqa: E402
from concourse.tile import TileContext  # noqa: E402
from concourse.tile_rust import add_dep_helper  # noqa: E402

BF16 = ml_dtypes.bfloat16
FP8 = ml_dtypes.float8_e4m3

# ----------------------------------------------------------------------------
# Problem configuration (hardcoded for nn_Encoder_17386027614431)
# ----------------------------------------------------------------------------
N_NODES = 20000
N_CORES = 8
T = 128
NT = 20                  # dst tiles per core
SHARD = NT * T           # 2560
NTAB = N_CORES * SHARD   # 20480 table rows
D0 = 256
DL = [256, 128, 64]      # per-layer output dims
HT_A = 4                 # tiles in AllGather chunk a
JCUT = 11                # tiles >= JCUT are cold: L2/L3 edges via gather
HALF_A = HT_A * T
HALF_B = SHARD - HALF_A
SA_TILES = N_CORES * HT_A          # 32 src tiles in chunk-a table
HOTB = JCUT - HT_A                 # hot b1 tiles per core (7)
NCOLD = NT - JCUT                  # cold tiles per core (9)
DPAD = [256, 128, 128]             # table row widths (L3 padded)
CCHUNK = 512                       # psum bank columns (f32)
NCH = SHARD // CCHUNK              # 5 feature-major column chunks
# AG chunk tile ranges per layer
AGCH = [
    [(0, HT_A), (HT_A, NT)],
    [(0, HT_A), (HT_A, JCUT), (JCUT, NT)],
    [(0, HT_A), (HT_A, JCUT), (JCUT, NT)],
]


def _ru16(x):
    return (int(x) + 15) // 16 * 16


def _offsets(cnt2d):
    flat = [c for row in cnt2d for c in row]
    nbl = [(c + T - 1) // T for c in flat]
    boff, ioff = [], []
    ob = oi = 0
    for c, nb in zip(flat, nbl):
        boff.append(ob)
        ioff.append(oi)
        ob += nb
        oi += c // 16
    return nbl, boff, ioff, ob, oi


def _build_nc(CNT, CNT2, apply_b1, apply_b2, apply_b3):
    f32 = mybir.dt.float32
    bf16 = mybir.dt.bfloat16
    fp8 = mybir.dt.float8e4
    i16 = mybir.dt.int16
    mult = mybir.AluOpType.mult
    add = mybir.AluOpType.add
    relu = mybir.ActivationFunctionType.Relu
    fcopy = mybir.ActivationFunctionType.Copy

    nbl_f, boff_f, ioff_f, totblk, idxcols = _offsets(CNT)
    nbl = [nbl_f[:NT], nbl_f[NT:]]
    boff = [boff_f[:NT], boff_f[NT:]]
    ioff = [ioff_f[:NT], ioff_f[NT:]]
    nbl2, boff2, ioff2, totblk2, idxcols2 = _offsets([CNT2])
    maxnb = max(max(nbl[0]), max(nbl[1]), max(nbl2))

    nc = bacc.Bacc("TRN2", num_devices=N_CORES, num_swdge_queues=4)

    # ---- kernel I/O ----
    xt = nc.dram_tensor("xt", [D0, SHARD], bf16, kind="ExternalInput")
    w1 = nc.dram_tensor("w1", [D0, DL[0]], bf16, kind="ExternalInput")
    w2 = nc.dram_tensor("w2", [DL[0], DL[1]], bf16, kind="ExternalInput")
    w3 = nc.dram_tensor("w3", [DL[1], DL[2]], bf16, kind="ExternalInput")
    brep1 = nc.dram_tensor("brep1", [T, DL[0]], f32, kind="ExternalInput")
    b2col = nc.dram_tensor("b2col", [T, 1], f32, kind="ExternalInput")
    b3col = nc.dram_tensor("b3col", [T, 1], f32, kind="ExternalInput")
    dis = nc.dram_tensor("dis", [T, NT], f32, kind="ExternalInput")
    dis2 = nc.dram_tensor("dis2", [T, NT], f32, kind="ExternalInput")
    disrow = nc.dram_tensor("disrow", [T, SHARD], f32, kind="ExternalInput")
    disrow2 = nc.dram_tensor("disrow2", [T, SHARD], f32,
                             kind="ExternalInput")
    identb = nc.dram_tensor("identb", [T, T], bf16, kind="ExternalInput")
    identf = nc.dram_tensor("identf", [T, T], f32, kind="ExternalInput")
    idx = nc.dram_tensor("idx", [T, idxcols], i16, kind="ExternalInput")
    sel = nc.dram_tensor("sel", [T, totblk * T], fp8, kind="ExternalInput")
    idx2 = nc.dram_tensor("idx2", [T, idxcols2], i16, kind="ExternalInput")
    sel2 = nc.dram_tensor("sel2", [T, totblk2 * T], fp8,
                          kind="ExternalInput")
    smat = nc.dram_tensor("smat", [NTAB, SHARD], fp8, kind="ExternalInput")
    out = nc.dram_tensor("out", [SHARD, DL[2]], f32, kind="ExternalOutput")

    # ---- internal DRAM for collectives (per layer, per AG chunk) ----
    agin, agout = [], []
    for l in range(3):
        ai, ao = [], []
        for k, (j0, j1) in enumerate(AGCH[l]):
            rows = (j1 - j0) * T
            ai.append(nc.dram_tensor(f"agin{l}_{k}", [rows, DPAD[l]], bf16))
            ao.append(nc.dram_tensor(
                f"agout{l}_{k}", [N_CORES * rows, DPAD[l]], bf16,
                addr_space="Shared"))
        agin.append(ai)
        agout.append(ao)
    rg = [list(range(N_CORES))]

    with TileContext(nc) as tc:
        nc.gpsimd.load_library(_mlp_lib)

        with (
            tc.tile_pool(name="const", bufs=1) as cpool,
            tc.tile_pool(name="sb", bufs=4) as sbpool,        # S stream
            tc.tile_pool(name="zsb", bufs=2) as zspool,       # Z stationary
            tc.tile_pool(name="selp", bufs=3) as selpool,
            tc.tile_pool(name="hp", bufs=2) as hpool,
            tc.tile_pool(name="htp", bufs=3) as htpool,
            tc.tile_pool(name="tmp", bufs=3) as tpool,
            tc.tile_pool(name="zbp", bufs=3) as zbpool,
            tc.tile_pool(name="ps_z", bufs=1, space="PSUM") as ps_z,
            tc.tile_pool(name="ps_agg", bufs=1, space="PSUM") as ps_agg,
            tc.tile_pool(name="ps_t", bufs=1, space="PSUM") as ps_t,
            tc.tile_pool(name="ps_fm", bufs=1, space="PSUM") as ps_fm,
        ):
            # ---- constants (xt/w1/dis first so Z1 starts immediately) ----
            def load_const(dram_h, shape, dtype):
                t = cpool.tile(shape, dtype, tag=f"c_{dram_h.name}")
                nc.sync.dma_start(out=t[:, :], in_=dram_h.ap())
                return t

            def load_const_chunked(dram_h, inner, dtype):
                cs = dram_h.shape[0] // T
                t = cpool.tile([T, cs * inner], dtype, tag=f"c_{dram_h.name}")
                nc.sync.dma_start(
                    out=t.rearrange("p (c n) -> p c n", c=cs),
                    in_=dram_h.ap().rearrange("(c p) n -> p c n", p=T),
                )
                return t

            xt_sb = load_const_chunked(xt, SHARD, bf16)
            w1_sb = load_const_chunked(w1, DL[0], bf16)
            dis_sb = load_const(dis, [T, NT], f32)
            dis2_sb = load_const(dis2, [T, NT], f32)
            identb_sb = load_const(identb, [T, T], bf16)
            idx_sb = load_const(idx, [T, idxcols], i16)
            idx2_sb = load_const(idx2, [T, idxcols2], i16)
            w2_sb = load_const_chunked(w2, DL[1], bf16)
            w3_sb = load_const(w3, [DL[1], DL[2]], bf16)
            brep1_sb = load_const(brep1, [T, DL[0]], f32)
            b2_sb = load_const(b2col, [T, 1], f32)
            b3_sb = load_const(b3col, [T, 1], f32)
            disrow_sb = load_const(disrow, [T, SHARD], f32)
            disrow2_sb = load_const(disrow2, [T, SHARD], f32)
            identf_sb = load_const(identf, [T, T], f32)

            # persistent buffers
            gbuf = [cpool.tile([T, maxnb * DL[0]], bf16, tag=f"g{i}",
                               name=f"gbuf{i}") for i in range(3)]
            for g in gbuf:
                nc.vector.memset(g[:, :], 0.0)
            zb1 = [cpool.tile([T, DL[0]], bf16, tag=f"zb1_{j}",
                              name=f"zb1_{j}") for j in range(NT)]
            acc = cpool.tile([T, NT * DL[0]], f32, tag="acc")
            h2fm = cpool.tile([T, SHARD], bf16, tag="h2fm")
            outfm = cpool.tile([T, SHARD], f32, tag="outfm")
            fm = [ps_fm.tile([T, CCHUNK], f32, tag=f"fm{c}",
                             name=f"fm{c}") for c in range(NCH)]
            nb2max = max(nbl2)
            cbuf = [cpool.tile([T, nb2max * DPAD[1]], bf16, tag=f"cb{j}",
                               name=f"cbuf{j}") for j in range(NT)]
            for cb in cbuf:
                nc.vector.memset(cb[:, :], 0.0)

            agin_v = [[agin[l][k].ap().rearrange("(n p) d -> p n d", p=T)
                       for k in range(len(AGCH[l]))] for l in range(3)]
            agout_v = [[agout[l][k].ap().rearrange("(n p) d -> p n d", p=T)
                        for k in range(len(AGCH[l]))] for l in range(3)]
            smat_v = smat.ap().rearrange("(s p) d -> p s d", p=T)
            out_v = out.ap().rearrange("(n p) d -> p n d", p=T)

            ag_insts = [[None] * len(AGCH[l]) for l in range(3)]
            agin_dmas = [[[] for _ in AGCH[l]] for l in range(3)]

            def z_store(l, j, zb):
                for k, (j0, j1) in enumerate(AGCH[l]):
                    if j0 <= j < j1:
                        break
                d = nc.sync.dma_start(
                    out=agin_v[l][k][:, j - j0, :], in_=zb[:, :])
                agin_dmas[l][k].append(d)

            def issue_ag(l, k):
                cc = nc.gpsimd.collective_compute(
                    "AllGather",
                    mybir.AluOpType.bypass,
                    replica_groups=rg,
                    ins=[agin[l][k].ap().opt()],
                    outs=[agout[l][k].ap().opt()],
                )
                for d in agin_dmas[l][k]:
                    add_dep_helper(cc.ins, d.ins, reason=f"ag{l}.{k}")
                ag_insts[l][k] = cc

            # ================= Layer 1: Z1 = (dis*x) @ W1 ==================
            # two tiles share one [T,512] psum bank; copies on scalar engine
            for jp in range(NT // 2):
                zp = ps_z.tile([T, 2 * DL[0]], f32, tag="zp")
                for half in range(2):
                    j = 2 * jp + half
                    o = half * DL[0]
                    for c in range(2):
                        nc.tensor.matmul(
                            zp[:, o:o + DL[0]],
                            xt_sb[:, c * SHARD + j * T:
                                  c * SHARD + (j + 1) * T],
                            w1_sb[:, c * DL[0]:(c + 1) * DL[0]],
                            start=(c == 0), stop=(c == 1),
                        )
                for half in range(2):
                    j = 2 * jp + half
                    nc.scalar.activation(
                        zb1[j][:, :], zp[:, half * DL[0]:(half + 1) * DL[0]],
                        fcopy)
                    z_store(0, j, zb1[j])
                if 2 * jp + 1 == HT_A - 1:
                    issue_ag(0, 0)
            issue_ag(0, 1)

            # ---- L1 gather helper ----
            gq = [0]

            def gather_group(h, j, gslot):
                cnt = CNT[h][j]
                nb = nbl[h][j]
                gt3 = gbuf[gslot][:, :nb * DL[0]].rearrange(
                    "p (n d) -> p n d", d=DL[0])
                g = nc.gpsimd.dma_gather(
                    gt3,
                    agout[0][h].ap(),
                    idx_sb[:, ioff[h][j]:ioff[h][j] + cnt // 16],
                    cnt, cnt, DL[0],
                    single_packet=False,
                    queue_num=gq[0] % 4,
                )
                gq[0] += 1
                add_dep_helper(g.ins, ag_insts[0][h].ins, reason="g ag")
                st = selpool.tile([T, maxnb * T], fp8, tag="sel")
                nc.sync.dma_start(
                    out=st[:, :nb * T],
                    in_=sel[:, boff[h][j] * T:(boff[h][j] + nb) * T])
                return gt3, st, nb

            # ---- cold gathers / matmuls for L2/L3 ----
            def cold_gathers(l):
                for j in range(NT):
                    cnt = CNT2[j]
                    gt3 = cbuf[j][:, :nbl2[j] * DPAD[l]].rearrange(
                        "p (n d) -> p n d", d=DPAD[l])
                    g = nc.gpsimd.dma_gather(
                        gt3,
                        agout[l][2].ap(),
                        idx2_sb[:, ioff2[j]:ioff2[j] + cnt // 16],
                        cnt, cnt, DPAD[l],
                        single_packet=False,
                        queue_num=gq[0] % 4,
                    )
                    gq[0] += 1
                    add_dep_helper(g.ins, ag_insts[l][2].ins, reason="cg ag")

            def cold_matmuls_chunk(l, c):
                """Fold cold edges of dst tiles 4c..4c+3 into fm[c]; the last
                one closes the accumulation group."""
                d_el = DL[l]
                for j in range(4 * c, 4 * c + 4):
                    nb = nbl2[j]
                    gt3 = cbuf[j][:, :nb * DPAD[l]].rearrange(
                        "p (n d) -> p n d", d=DPAD[l])
                    st = selpool.tile([T, maxnb * T], fp8, tag="sel")
                    nc.sync.dma_start(
                        out=st[:, :nb * T],
                        in_=sel2[:, boff2[j] * T:(boff2[j] + nb) * T])
                    r = (j % 4) * T
                    for b in range(nb):
                        nc.tensor.matmul(
                            fm[c][:d_el, r:r + T],
                            gt3[:, b, :d_el],
                            st[:, b * T:(b + 1) * T],
                            start=False,
                            stop=(j % 4 == 3 and b == nb - 1),
                            skip_group_check=True)

            # ---- hot scatter: chunk-a tiles then per-core b1 stripes; S
            # fetched two tiles per DMA on the Activation HWDGE queue. ----
            fetch_plan = []           # (smat_row0, ntiles, ag chunk)
            for g2 in range(SA_TILES // 2):
                fetch_plan.append((2 * g2, 2, 0))
            for core in range(N_CORES):
                base = SA_TILES + core * HOTB
                k = 0
                while k < HOTB:
                    n = min(2, HOTB - k)
                    fetch_plan.append((base + k, n, 1))
                    k += n
            hot_steps = []            # (fetch_idx, k_in_fetch)
            for fi, (r0, n, ch) in enumerate(fetch_plan):
                for k in range(n):
                    hot_steps.append((fi, k))
            n_hot = len(hot_steps)

            scat = {"pos": 0, "zsb": None, "stile": None}

            def scatter_steps(l, n, limit):
                d_el = DL[l]
                dp = DPAD[l]
                while n > 0 and scat["pos"] < limit:
                    pos = scat["pos"]
                    fi, k = hot_steps[pos]
                    r0, fn, ch = fetch_plan[fi]
                    if pos < SA_TILES:
                        if pos % 8 == 0:
                            zsb = zspool.tile([T, 8 * dp], bf16,
                                              tag=f"zsa{l}")
                            d = nc.sync.dma_start(
                                out=zsb.rearrange("p (n d) -> p n d", d=dp),
                                in_=agout_v[l][0][:, pos:pos + 8, :])
                            add_dep_helper(d.ins, ag_insts[l][0].ins,
                                           reason="zs ag")
                            scat["zsb"] = zsb
                        zk = pos % 8
                    else:
                        p = pos - SA_TILES
                        if p % HOTB == 0:
                            core = p // HOTB
                            zsb = zspool.tile([T, HOTB * dp], bf16,
                                              tag=f"zsb{l}")
                            d = nc.sync.dma_start(
                                out=zsb.rearrange("p (n d) -> p n d", d=dp),
                                in_=agout_v[l][1][:, core * HOTB:
                                                  (core + 1) * HOTB, :])
                            add_dep_helper(d.ins, ag_insts[l][1].ins,
                                           reason="zs ag")
                            scat["zsb"] = zsb
                        zk = p % HOTB
                    if k == 0:
                        stile = sbpool.tile([T, 2 * SHARD], fp8, tag="sm")
                        nc.scalar.dma_start(
                            out=stile[:, :fn * SHARD].rearrange(
                                "p (n d) -> p n d", d=SHARD),
                            in_=smat_v[:, r0:r0 + fn, :])
                        scat["stile"] = stile
                    stile = scat["stile"]
                    zsb = scat["zsb"]
                    for c in range(NCH):
                        nc.tensor.matmul(
                            fm[c][:d_el, :],
                            zsb[:, zk * dp:zk * dp + d_el],
                            stile[:, k * SHARD + c * CCHUNK:
                                  k * SHARD + (c + 1) * CCHUNK],
                            start=(pos == 0), stop=False,
                            skip_group_check=True)
                    scat["pos"] = pos + 1
                    n -= 1

            # ================= L1 phase A (src chunk a) ====================
            for j in range(NT):
                gt3, st, nb = gather_group(0, j, j % 3)
                ps = ps_agg.tile([T, DL[0]], f32, tag="agg")
                nc.tensor.matmul(ps[:, :], identb_sb[:, :], zb1[j][:, :],
                                 start=True, stop=False)
                for b in range(nb):
                    nc.tensor.matmul(
                        ps[:, :], st[:, b * T:(b + 1) * T], gt3[:, b, :],
                        start=False, stop=(b == nb - 1))
                nc.scalar.activation(
                    acc[:, j * DL[0]:(j + 1) * DL[0]], ps[:, :], fcopy)

            # ================= L1 phase B + post + Z2 + AG2 ================
            for j in range(NT):
                gt3, st, nb = gather_group(1, j, j % 3)
                ps = ps_agg.tile([T, DL[0]], f32, tag="agg")
                for b in range(nb):
                    nc.tensor.matmul(
                        ps[:, :], st[:, b * T:(b + 1) * T], gt3[:, b, :],
                        start=(b == 0), stop=(b == nb - 1))
                u = tpool.tile([T, DL[0]], f32, tag="post")
                nc.vector.tensor_tensor(
                    u[:, :], ps[:, :], acc[:, j * DL[0]:(j + 1) * DL[0]], add)
                h1 = hpool.tile([T, DL[0]], bf16, tag="h1")
                if apply_b1:
                    # generic path: h1 = relu(dis*u + b1); zb2 prescaled later
                    t1 = tpool.tile([T, DL[0]], f32, tag="post")
                    nc.vector.tensor_scalar(
                        t1[:, :], u[:, :], dis_sb[:, j:j + 1], None, mult)
                    t2 = tpool.tile([T, DL[0]], f32, tag="post")
                    nc.vector.tensor_tensor(t2[:, :], t1[:, :],
                                            brep1_sb[:, :], add)
                    nc.scalar.activation(h1[:, :], t2[:, :], relu)
                else:
                    # h1 = dis*relu(dis*u) = relu(dis^2*u): Z2 comes out
                    # pre-scaled for the next aggregation
                    nc.scalar.activation(h1[:, :], u[:, :], relu,
                                         scale=dis2_sb[:, j:j + 1])
                zp2 = ps_z.tile([T, 2 * DL[0]], f32, tag="zp")
                for c in range(2):
                    tp = ps_t.tile([T, T], bf16, tag="tp")
                    nc.tensor.matmul(tp[:, :], h1[:, c * T:(c + 1) * T],
                                     identb_sb[:, :], is_transpose=True)
                    htc = htpool.tile([T, T], bf16, tag="ht")
                    nc.scalar.activation(htc[:, :], tp[:, :], fcopy)
                    nc.tensor.matmul(
                        zp2[:, :DL[1]], htc[:, :],
                        w2_sb[:, c * DL[1]:(c + 1) * DL[1]],
                        start=(c == 0), stop=(c == 1))
                zb2 = zbpool.tile([T, DL[1]], bf16, tag="zb2")
                if apply_b1:
                    nc.vector.tensor_scalar(
                        zb2[:, :], zp2[:, :DL[1]], dis_sb[:, j:j + 1],
                        None, mult)
                else:
                    nc.scalar.activation(zb2[:, :], zp2[:, :DL[1]], fcopy)
                z_store(1, j, zb2)
                if j == HT_A - 1:
                    issue_ag(1, 0)
                if j == JCUT - 1:
                    issue_ag(1, 1)
                if j == NT - 1:
                    issue_ag(1, 2)
                if j >= HT_A:
                    lim = SA_TILES if j < JCUT else n_hot
                    scatter_steps(1, 3, lim)

            # ---- layer tails: finish aggregation, pipeline per chunk ----
            def layer_tail(l):
                last = l == 2
                cold_gathers(l)
                scatter_steps(l, n_hot, n_hot)
                for c in range(NCH):
                    cold_matmuls_chunk(l, c)
                    d_el = DL[l]
                    t = tpool.tile([T, CCHUNK], f32, tag="fmpost")
                    if not last:
                        drow = disrow_sb if apply_b2 else disrow2_sb
                        nc.vector.tensor_tensor(
                            t[:d_el, :], fm[c][:d_el, :],
                            drow[:d_el, c * CCHUNK:(c + 1) * CCHUNK], mult)
                        nc.scalar.activation(
                            h2fm[:, c * CCHUNK:(c + 1) * CCHUNK],
                            t[:d_el, :], relu, bias=b2_sb[:, :])
                        for j in range(4 * c, 4 * c + 4):
                            zp3 = ps_z.tile([T, 2 * DL[0]], f32, tag="zp")
                            nc.tensor.matmul(
                                zp3[:, :DL[2]], h2fm[:, j * T:(j + 1) * T],
                                w3_sb[:, :], start=True, stop=True)
                            zb3 = zbpool.tile([T, DPAD[2]], bf16, tag="zb3")
                            if apply_b2:
                                nc.vector.tensor_scalar(
                                    zb3[:, :DL[2]], zp3[:, :DL[2]],
                                    dis_sb[:, j:j + 1], None, mult)
                            else:
                                nc.scalar.activation(
                                    zb3[:, :DL[2]], zp3[:, :DL[2]], fcopy)
                            z_store(2, j, zb3)
                            if j == HT_A - 1:
                                issue_ag(2, 0)
                            if j == JCUT - 1:
                                issue_ag(2, 1)
                            if j == NT - 1:
                                issue_ag(2, 2)
                    else:
                        nc.vector.tensor_tensor(
                            t[:d_el, :], fm[c][:d_el, :],
                            disrow_sb[:d_el, c * CCHUNK:(c + 1) * CCHUNK],
                            mult)
                        if apply_b3:
                            nc.scalar.activation(
                                outfm[:DL[2], c * CCHUNK:(c + 1) * CCHUNK],
                                t[:DL[2], :], fcopy, bias=b3_sb[:DL[2], :])
                        else:
                            nc.scalar.activation(
                                outfm[:DL[2], c * CCHUNK:(c + 1) * CCHUNK],
                                t[:DL[2], :], fcopy)
                        for j in range(4 * c, 4 * c + 4):
                            tpf = ps_z.tile([T, 2 * DL[0]], f32, tag="zp")
                            nc.tensor.matmul(
                                tpf[:, :DL[2]],
                                outfm[:DL[2], j * T:(j + 1) * T],
                                identf_sb[:DL[2], :DL[2]],
                                is_transpose=True)
                            ot = htpool.tile([T, DL[2]], f32, tag="ot")
                            nc.scalar.activation(ot[:, :], tpf[:, :DL[2]],
                                                 fcopy)
                            nc.sync.dma_start(out=out_v[:, j, :],
                                              in_=ot[:, :])

            layer_tail(1)
            scat["pos"] = 0
            layer_tail(2)

    nc.compile()
    return nc


# ----------------------------------------------------------------------------
# Host-side preprocessing
# ----------------------------------------------------------------------------
def _band_node_order(outdeg, indeg):
    by_out = np.argsort(-outdeg, kind="stable")
    node_order = -np.ones(NTAB, np.int64)
    new_pos = np.zeros(N_NODES, np.int64)
    band_sz = N_CORES * T
    for k in range(NT):
        band = by_out[k * band_sz:(k + 1) * band_sz]
        band = band[np.argsort(-indeg[band], kind="stable")]
        fill = np.zeros(N_CORES, np.int64)
        b = 0
        direction = 1
        for node in band:
            pos = b * SHARD + k * T + fill[b]
            node_order[pos] = node
            new_pos[node] = pos
            fill[b] += 1
            b += direction
            if b == N_CORES:
                b = N_CORES - 1
                direction = -1
            elif b < 0:
                b = 0
                direction = 1
    return node_order, new_pos


def _group_pack(core_s, grp, ngrp, row_s, slot_s, CNT_flat, ioff_flat,
                boff_flat, mult_s=None):
    grp_start = np.zeros(N_CORES * ngrp + 1, np.int64)
    np.add.at(grp_start, core_s * ngrp + grp + 1, 1)
    grp_start = np.cumsum(grp_start)
    rank = np.arange(len(grp)) - grp_start[core_s * ngrp + grp]
    cnt_np = np.array(CNT_flat)
    ioff_np = np.array(ioff_flat)
    boff_np = np.array(boff_flat)
    epos = ioff_np[grp] * 16 + rank
    blk = boff_np[grp] + rank // T
    lane = rank % T
    idxcols = int(ioff_np[-1] + cnt_np[-1] // 16)
    totblk = int(boff_np[-1] + (cnt_np[-1] + T - 1) // T)
    idx_cores, sel_cores = [], []
    for c in range(N_CORES):
        m = core_s == c
        flat = np.zeros(idxcols * 16, np.int16)
        flat[epos[m]] = row_s[m].astype(np.int16)
        wrapped = np.tile(flat.reshape(idxcols, 16).T, (8, 1))
        idx_cores.append(np.ascontiguousarray(wrapped.astype(np.int16)))
        selc = np.zeros((totblk, T, T), np.uint8)
        selc[blk[m], lane[m], slot_s[m]] = (
            1 if mult_s is None else mult_s[m])
        sel_cores.append(np.ascontiguousarray(
            selc.transpose(1, 0, 2).reshape(T, totblk * T)).astype(FP8))
    return idx_cores, sel_cores


def _preprocess(edge_index):
    src = np.asarray(edge_index[0], dtype=np.int64)
    dst = np.asarray(edge_index[1], dtype=np.int64)
    indeg = np.bincount(dst, minlength=N_NODES).astype(np.float64) + 1.0
    outdeg = np.bincount(src, minlength=N_NODES).astype(np.float64)
    dis_full = 1.0 / np.sqrt(indeg)

    node_order, new_pos = _band_node_order(outdeg, indeg)

    spos = new_pos[src]
    dpos = new_pos[dst]
    core = dpos // SHARD
    tile = (dpos % SHARD) // T
    slot = dpos % T
    shalf = ((spos % SHARD) >= HALF_A).astype(np.int64)
    srow_half = ((spos // SHARD) * np.where(shalf == 0, HALF_A, HALF_B)
                 + (spos % SHARD) - shalf * HALF_A)

    key = ((((core * 2 + shalf) * NT + tile) * 16384
            + srow_half) * T + slot)
    uniq, uidx, ucnt = np.unique(key, return_index=True, return_counts=True)
    core_u = core[uidx]
    shalf_u = shalf[uidx]
    tile_u = tile[uidx]
    slot_u = slot[uidx]
    srow_u = srow_half[uidx]
    counts = np.zeros((N_CORES, 2, NT), np.int64)
    np.add.at(counts, (core_u, shalf_u, tile_u), 1)
    CNT = [[max(16, _ru16(counts[:, h, j].max())) for j in range(NT)]
           for h in range(2)]
    CNT_flat = [c for row in CNT for c in row]
    _, boff_f, ioff_f, _, _ = _offsets(CNT)

    order = np.lexsort((slot_u, tile_u, shalf_u, core_u))
    g1 = shalf_u[order] * NT + tile_u[order]
    idx_cores, sel_cores = _group_pack(
        core_u[order], g1, 2 * NT, srow_u[order], slot_u[order],
        CNT_flat, ioff_f, boff_f, mult_s=ucnt[order])

    # ---- cold edges (src tile >= JCUT -> AG chunk 2) for L2/L3 ----
    loop_pos = new_pos[node_order[node_order >= 0]]
    s_all = np.concatenate([spos, loop_pos])
    d_all = np.concatenate([dpos, loop_pos])
    stile_all = (s_all % SHARD) // T
    cold_m = stile_all >= JCUT
    sc = s_all[cold_m]
    dc = d_all[cold_m]
    ccore = dc // SHARD
    ctile = (dc % SHARD) // T
    cslot = dc % T
    crow = (sc // SHARD) * (NCOLD * T) + (sc % SHARD) - JCUT * T
    key2 = (((ccore * NT + ctile) * 32768 + crow) * T + cslot)
    uq2, ui2, uc2 = np.unique(key2, return_index=True, return_counts=True)
    ccore_u = ccore[ui2]
    ctile_u = ctile[ui2]
    cslot_u = cslot[ui2]
    crow_u = crow[ui2]
    counts2 = np.zeros((N_CORES, NT), np.int64)
    np.add.at(counts2, (ccore_u, ctile_u), 1)
    CNT2 = [max(16, _ru16(counts2[:, j].max())) for j in range(NT)]
    _, boff2_f, ioff2_f, _, _ = _offsets([CNT2])
    order2 = np.lexsort((cslot_u, ctile_u, ccore_u))
    idx2_cores, sel2_cores = _group_pack(
        ccore_u[order2], ctile_u[order2], NT, crow_u[order2],
        cslot_u[order2], CNT2, ioff2_f, boff2_f, mult_s=uc2[order2])

    # ---- S matrices: hot srcs only, rows in [a | b1-hot] order ----
    hot_m = ~cold_m
    sh = s_all[hot_m]
    dh = d_all[hot_m]
    sh_tile = (sh % SHARD) // T
    sh_core = sh // SHARD
    sh_off = sh % T
    in_a = sh_tile < HT_A
    srow_glob = np.where(
        in_a,
        sh_core * HALF_A + sh_tile * T + sh_off,
        N_CORES * HALF_A + sh_core * (HOTB * T)
        + (sh_tile - HT_A) * T + sh_off)
    dcore_h = dh // SHARD
    dloc_h = dh % SHARD
    smat_cores = []
    for c in range(N_CORES):
        m = dcore_h == c
        S = np.zeros((NTAB, SHARD), np.uint8)
        np.add.at(S, (srow_glob[m], dloc_h[m]), 1)
        smat_cores.append(S.astype(FP8))

    dis_cores, disrow_cores = [], []
    for c in range(N_CORES):
        slots = node_order[c * SHARD:(c + 1) * SHARD]
        dis_c = np.where(slots >= 0, dis_full[np.maximum(slots, 0)], 0.0)
        dis_cores.append(np.ascontiguousarray(
            dis_c.reshape(NT, T).T).astype(np.float32))
        disrow_cores.append(np.ascontiguousarray(
            np.tile(dis_c[None, :], (T, 1))).astype(np.float32))

    return (idx_cores, sel_cores, idx2_cores, sel2_cores, dis_cores,
            disrow_cores, smat_cores, CNT, CNT2, node_order)


def _make_in_maps(x, W1, b1, W2, b2, W3, b3, edge_index):
    (idx_cores, sel_cores, idx2_cores, sel2_cores, dis_cores, disrow_cores,
     smat_cores, CNT, CNT2, node_order) = _preprocess(edge_index)

    x = np.asarray(x, np.float32)
    w1b = np.asarray(W1, np.float32).astype(BF16)
    w2b = np.asarray(W2, np.float32).astype(BF16)
    w3b = np.asarray(W3, np.float32).astype(BF16)
    b1f = np.asarray(b1, np.float32)
    b2f = np.asarray(b2, np.float32)
    b3f = np.asarray(b3, np.float32)
    brep1 = np.tile(b1f, (T, 1))
    b2col = np.zeros((T, 1), np.float32)
    b2col[:DL[1], 0] = b2f
    b3col = np.zeros((T, 1), np.float32)
    b3col[:DL[2], 0] = b3f
    identb = np.eye(T, dtype=BF16)
    identf = np.eye(T, dtype=np.float32)
    apply_b1 = bool(np.any(b1f))
    apply_b2 = bool(np.any(b2f))
    apply_b3 = bool(np.any(b3f))

    in_maps = []
    for c in range(N_CORES):
        slots = node_order[c * SHARD:(c + 1) * SHARD]
        xs = np.where((slots >= 0)[:, None], x[np.maximum(slots, 0)], 0.0)
        xs = xs * disrow_cores[c][0][:, None]
        in_maps.append({
            "xt": np.ascontiguousarray(xs.T.astype(np.float32)).astype(BF16),
            "w1": w1b, "w2": w2b, "w3": w3b,
            "brep1": brep1, "b2col": b2col, "b3col": b3col,
            "dis": dis_cores[c], "dis2": dis_cores[c] ** 2,
            "disrow": disrow_cores[c], "disrow2": disrow_cores[c] ** 2,
            "identb": identb, "identf": identf,
            "idx": idx_cores[c], "sel": sel_cores[c],
            "idx2": idx2_cores[c], "sel2": sel2_cores[c],
            "smat": smat_cores[c],
        })
    return in_maps, CNT, CNT2, node_order, apply_b1, apply_b2, apply_b3


_NC_CACHE = {}


def kernel_with_results(x, W1, b1, W2, b2, W3, b3, edge_index, trace=False):
    (in_maps, CNT, CNT2, node_order, apply_b1, apply_b2,
     apply_b3) = _make_in_maps(x, W1, b1, W2, b2, W3, b3, edge_index)
    key = (tuple(CNT[0]), tuple(CNT[1]), tuple(CNT2), apply_b1, apply_b2,
           apply_b3)
    if key not in _NC_CACHE:
        _NC_CACHE[key] = _build_nc(CNT, CNT2, apply_b1, apply_b2, apply_b3)
    nc = _NC_CACHE[key]
    res = run_bass_kernel_spmd(
        nc, in_maps, core_ids=list(range(N_CORES)), trace=trace)
    rows = np.concatenate(
        [np.asarray(res.results[c]["out"]) for c in range(N_CORES)], axis=0)
    full = np.zeros((N_NODES, rows.shape[1]), np.float32)
    real = node_order >= 0
    full[node_order[real]] = rows[real]
    return full, res


def kernel(x, W1, b1, W2, b2, W3, b3, edge_index):
    full, _ = kernel_with_results(x, W1, b1, W2, b2, W3, b3, edge_index)
    return full


# revision 28
# speedup vs baseline: 1.0778x; 1.0701x over previous
"""GCN (3-layer, PyG GCNConv-style) forward pass on 8 Trainium2 NeuronCores.

Architecture v5 (gather L1 + hot/cold split PE-scatter L2/L3, chunked AG):
  - Nodes are assigned to tiles by OUT-degree bands (band k -> tile k on
    every core; within a band, snake-dealt by IN-degree across cores), so
    tile index correlates with out-degree.  Tiles >= JCUT hold the
    lowest-out-degree nodes ("cold"), the rest are "hot".
  - Z_l = dis * (H @ Wl) computed per core and AllGathered in chunks:
    layer 1 in halves (a = tiles 0..3, b = 4..19); layers 2/3 in three
    chunks aligned with the hot/cold boundary (a = 0..3, b1 = 4..JCUT-1
    hot, b2 = JCUT..19 cold) so the hot scatter can begin as soon as b1
    lands while cold gathers wait for b2.  Layer-3 rows padded to 128
    (gather needs 256B rows).
  - Layer 1 aggregation: SWDGE dma_gather + fp8-selector matmuls (gather
    costs ~7 ns/row of GpSimd regardless of width -> it handles the widest
    layer).  Self-loops enter via an identity matmul on the local Z tile;
    gather calls use exact per-group counts on 4 SWDGE queues.
  - Layers 2/3, hot source tiles: PE-scatter.  Z_s is stationary, a 0/1
    adjacency slice S_s [128 src x 2560 dst] (fp8, self-loops included)
    streams as the moving operand (fetched two tiles per DMA on the
    Activation engine's HWDGE queue), accumulating a feature-major PSUM
    [d x 2560] in five 512-col bank chunks.
  - Layers 2/3, cold source tiles: edges dma_gathered (GpSimd idles during
    scatter otherwise) and folded into the same PSUM via per-dst-tile
    selector matmuls with the gathered block stationary.
  - The layer tail is pipelined per 512-col chunk: close chunk c (cold
    matmuls) -> feature-major post -> next-layer GEMM for its 4 node tiles
    -> AllGather chunk fires as soon as its tiles are stored.
  - Post-ops run feature-major (dis as a replicated row, bias per
    partition); the next GEMM consumes H_fm directly as lhsT; the final
    output is PE-transposed back to node-major.
"""

import sys

import numpy as np

sys.path.insert(0, "/opt/trn_rl_repo")

import ml_dtypes  # noqa: E402

import concourse.bass as bass  # noqa: E402
import concourse.bacc as bacc  # noqa: E402
import concourse.mybir as mybir  # noqa: E402
from concourse.bass_utils import run_bass_kernel_spmd  # noqa: E402
from concourse.library_config import mlp as _mlp_lib  # noqa: E402
from concourse.tile import TileContext  # noqa: E402
from concourse.tile_rust import add_dep_helper  # noqa: E402

BF16 = ml_dtypes.bfloat16
FP8 = ml_dtypes.float8_e4m3

# ----------------------------------------------------------------------------
# Problem configuration (hardcoded for nn_Encoder_17386027614431)
# ----------------------------------------------------------------------------
N_NODES = 20000
N_CORES = 8
T = 128
NT = 20                  # dst tiles per core
SHARD = NT * T           # 2560
NTAB = N_CORES * SHARD   # 20480 table rows
D0 = 256
DL = [256, 128, 64]      # per-layer output dims
HT_A = 4                 # tiles in AllGather chunk a
JCUT = 11                # L2: tiles >= JCUT are cold (edges via gather)
JCUT3 = 8                # L3: more cold tiles (GpSimd idles in L3 phase)
JC = {1: JCUT, 2: JCUT3}
HALF_A = HT_A * T
HALF_B = SHARD - HALF_A
SA_TILES = N_CORES * HT_A          # 32 src tiles in chunk-a table
HOTB = JCUT - HT_A                 # hot b1 tiles per core in smat layout
B1N = {1: JCUT - HT_A, 2: JCUT3 - HT_A}   # b1 tiles per core per layer
NCOLDL = {1: NT - JCUT, 2: NT - JCUT3}
DPAD = [256, 128, 128]             # table row widths (L3 padded)
CCHUNK = 512                       # psum bank columns (f32)
NCH = SHARD // CCHUNK              # 5 feature-major column chunks
# AG chunk tile ranges per layer
AGCH = [
    [(0, HT_A), (HT_A, NT)],
    [(0, HT_A), (HT_A, JCUT), (JCUT, NT)],
    [(0, HT_A), (HT_A, JCUT3), (JCUT3, NT)],
]


def _ru16(x):
    return (int(x) + 15) // 16 * 16


def _offsets(cnt2d):
    flat = [c for row in cnt2d for c in row]
    nbl = [(c + T - 1) // T for c in flat]
    boff, ioff = [], []
    ob = oi = 0
    for c, nb in zip(flat, nbl):
        boff.append(ob)
        ioff.append(oi)
        ob += nb
        oi += c // 16
    return nbl, boff, ioff, ob, oi


def _build_nc(CNT, CNTC, apply_b1, apply_b2, apply_b3):
    f32 = mybir.dt.float32
    bf16 = mybir.dt.bfloat16
    fp8 = mybir.dt.float8e4
    i16 = mybir.dt.int16
    mult = mybir.AluOpType.mult
    add = mybir.AluOpType.add
    relu = mybir.ActivationFunctionType.Relu
    fcopy = mybir.ActivationFunctionType.Copy

    nbl_f, boff_f, ioff_f, totblk, idxcols = _offsets(CNT)
    nbl = [nbl_f[:NT], nbl_f[NT:]]
    boff = [boff_f[:NT], boff_f[NT:]]
    ioff = [ioff_f[:NT], ioff_f[NT:]]
    nblc, boffc, ioffc, totblkc, idxcolsc = {}, {}, {}, {}, {}
    for l in (1, 2):
        nblc[l], boffc[l], ioffc[l], totblkc[l], idxcolsc[l] = _offsets(
            [CNTC[l]])
    maxnb = max(max(nbl[0]), max(nbl[1]), max(nblc[1]), max(nblc[2]))

    nc = bacc.Bacc("TRN2", num_devices=N_CORES, num_swdge_queues=4)

    # ---- kernel I/O ----
    xt = nc.dram_tensor("xt", [D0, SHARD], bf16, kind="ExternalInput")
    w1 = nc.dram_tensor("w1", [D0, DL[0]], bf16, kind="ExternalInput")
    w2 = nc.dram_tensor("w2", [DL[0], DL[1]], bf16, kind="ExternalInput")
    w3 = nc.dram_tensor("w3", [DL[1], DL[2]], bf16, kind="ExternalInput")
    brep1 = nc.dram_tensor("brep1", [T, DL[0]], f32, kind="ExternalInput")
    b2col = nc.dram_tensor("b2col", [T, 1], f32, kind="ExternalInput")
    b3col = nc.dram_tensor("b3col", [T, 1], f32, kind="ExternalInput")
    dis = nc.dram_tensor("dis", [T, NT], f32, kind="ExternalInput")
    dis2 = nc.dram_tensor("dis2", [T, NT], f32, kind="ExternalInput")
    disrow = nc.dram_tensor("disrow", [T, SHARD], bf16,
                            kind="ExternalInput")
    disrow2 = nc.dram_tensor("disrow2", [T, SHARD], bf16,
                             kind="ExternalInput")
    identb = nc.dram_tensor("identb", [T, T], bf16, kind="ExternalInput")
    identf = nc.dram_tensor("identf", [T, T], f32, kind="ExternalInput")
    idx = nc.dram_tensor("idx", [T, idxcols], i16, kind="ExternalInput")
    sel = nc.dram_tensor("sel", [T, totblk * T], fp8, kind="ExternalInput")
    idxc = {l: nc.dram_tensor(f"idxc{l}", [T, idxcolsc[l]], i16,
                              kind="ExternalInput") for l in (1, 2)}
    selc = {l: nc.dram_tensor(f"selc{l}", [T, totblkc[l] * T], fp8,
                              kind="ExternalInput") for l in (1, 2)}
    smat = nc.dram_tensor("smat", [NTAB, SHARD], fp8, kind="ExternalInput")
    out = nc.dram_tensor("out", [SHARD, DL[2]], f32, kind="ExternalOutput")

    # ---- internal DRAM for collectives (per layer, per AG chunk) ----
    agin, agout = [], []
    for l in range(3):
        ai, ao = [], []
        for k, (j0, j1) in enumerate(AGCH[l]):
            rows = (j1 - j0) * T
            ai.append(nc.dram_tensor(f"agin{l}_{k}", [rows, DPAD[l]], bf16))
            ao.append(nc.dram_tensor(
                f"agout{l}_{k}", [N_CORES * rows, DPAD[l]], bf16,
                addr_space="Shared"))
        agin.append(ai)
        agout.append(ao)
    rg = [list(range(N_CORES))]

    with TileContext(nc) as tc:
        nc.gpsimd.load_library(_mlp_lib)

        with (
            tc.tile_pool(name="const", bufs=1) as cpool,
            tc.tile_pool(name="sb", bufs=4) as sbpool,        # S stream
            tc.tile_pool(name="zsb", bufs=2) as zspool,       # Z stationary
            tc.tile_pool(name="selp", bufs=3) as selpool,
            tc.tile_pool(name="hp", bufs=2) as hpool,
            tc.tile_pool(name="htp", bufs=3) as htpool,
            tc.tile_pool(name="tmp", bufs=3) as tpool,
            tc.tile_pool(name="zbp", bufs=3) as zbpool,
            tc.tile_pool(name="ps_z", bufs=1, space="PSUM") as ps_z,
            tc.tile_pool(name="ps_agg", bufs=1, space="PSUM") as ps_agg,
            tc.tile_pool(name="ps_t", bufs=1, space="PSUM") as ps_t,
            tc.tile_pool(name="ps_fm", bufs=1, space="PSUM") as ps_fm,
        ):
            # ---- constants (xt/w1/dis first so Z1 starts immediately) ----
            def load_const(dram_h, shape, dtype):
                t = cpool.tile(shape, dtype, tag=f"c_{dram_h.name}")
                nc.sync.dma_start(out=t[:, :], in_=dram_h.ap())
                return t

            def load_const_chunked(dram_h, inner, dtype):
                cs = dram_h.shape[0] // T
                t = cpool.tile([T, cs * inner], dtype, tag=f"c_{dram_h.name}")
                nc.sync.dma_start(
                    out=t.rearrange("p (c n) -> p c n", c=cs),
                    in_=dram_h.ap().rearrange("(c p) n -> p c n", p=T),
                )
                return t

            xt_sb = load_const_chunked(xt, SHARD, bf16)
            w1_sb = load_const_chunked(w1, DL[0], bf16)
            dis_sb = load_const(dis, [T, NT], f32)
            dis2_sb = load_const(dis2, [T, NT], f32)
            identb_sb = load_const(identb, [T, T], bf16)
            idx_sb = load_const(idx, [T, idxcols], i16)
            idxc_sb = {l: load_const(idxc[l], [T, idxcolsc[l]], i16)
                       for l in (1, 2)}
            w2_sb = load_const_chunked(w2, DL[1], bf16)
            w3_sb = load_const(w3, [DL[1], DL[2]], bf16)
            brep1_sb = load_const(brep1, [T, DL[0]], f32)
            b2_sb = load_const(b2col, [T, 1], f32)
            b3_sb = load_const(b3col, [T, 1], f32)
            disrow_sb = load_const(disrow, [T, SHARD], bf16)
            disrow2_sb = load_const(disrow2, [T, SHARD], bf16)
            identf_sb = load_const(identf, [T, T], f32)

            # persistent buffers
            gbuf = [cpool.tile([T, maxnb * DL[0]], bf16, tag=f"g{i}",
                               name=f"gbuf{i}") for i in range(3)]
            for g in gbuf:
                nc.vector.memset(g[:, :], 0.0)
            zb1 = [cpool.tile([T, DL[0]], bf16, tag=f"zb1_{j}",
                              name=f"zb1_{j}") for j in range(NT)]
            acc = cpool.tile([T, NT * DL[0]], f32, tag="acc")
            h2fm = cpool.tile([T, SHARD], bf16, tag="h2fm")
            outfm = cpool.tile([T, SHARD], f32, tag="outfm")
            fm = [ps_fm.tile([T, CCHUNK], f32, tag=f"fm{c}",
                             name=f"fm{c}") for c in range(NCH)]
            nb2max = max(max(nblc[1]), max(nblc[2]))
            cbuf = [cpool.tile([T, nb2max * DPAD[1]], bf16, tag=f"cb{j}",
                               name=f"cbuf{j}") for j in range(NT)]
            for cb in cbuf:
                nc.vector.memset(cb[:, :], 0.0)

            agin_v = [[agin[l][k].ap().rearrange("(n p) d -> p n d", p=T)
                       for k in range(len(AGCH[l]))] for l in range(3)]
            agout_v = [[agout[l][k].ap().rearrange("(n p) d -> p n d", p=T)
                        for k in range(len(AGCH[l]))] for l in range(3)]
            smat_v = smat.ap().rearrange("(s p) d -> p s d", p=T)
            out_v = out.ap().rearrange("(n p) d -> p n d", p=T)

            ag_insts = [[None] * len(AGCH[l]) for l in range(3)]
            agin_dmas = [[[] for _ in AGCH[l]] for l in range(3)]

            def z_store(l, j, zb):
                for k, (j0, j1) in enumerate(AGCH[l]):
                    if j0 <= j < j1:
                        break
                d = nc.sync.dma_start(
                    out=agin_v[l][k][:, j - j0, :], in_=zb[:, :])
                agin_dmas[l][k].append(d)

            def issue_ag(l, k):
                cc = nc.gpsimd.collective_compute(
                    "AllGather",
                    mybir.AluOpType.bypass,
                    replica_groups=rg,
                    ins=[agin[l][k].ap().opt()],
                    outs=[agout[l][k].ap().opt()],
                )
                for d in agin_dmas[l][k]:
                    add_dep_helper(cc.ins, d.ins, reason=f"ag{l}.{k}")
                ag_insts[l][k] = cc

            # ================= Layer 1: Z1 = (dis*x) @ W1 ==================
            # two tiles share one [T,512] psum bank; copies on scalar engine
            for jp in range(NT // 2):
                zp = ps_z.tile([T, 2 * DL[0]], f32, tag="zp")
                for half in range(2):
                    j = 2 * jp + half
                    o = half * DL[0]
                    for c in range(2):
                        nc.tensor.matmul(
                            zp[:, o:o + DL[0]],
                            xt_sb[:, c * SHARD + j * T:
                                  c * SHARD + (j + 1) * T],
                            w1_sb[:, c * DL[0]:(c + 1) * DL[0]],
                            start=(c == 0), stop=(c == 1),
                        )
                for half in range(2):
                    j = 2 * jp + half
                    nc.scalar.activation(
                        zb1[j][:, :], zp[:, half * DL[0]:(half + 1) * DL[0]],
                        fcopy)
                    z_store(0, j, zb1[j])
                if 2 * jp + 1 == HT_A - 1:
                    issue_ag(0, 0)
            issue_ag(0, 1)

            # ---- L1 gather helper ----
            gq = [0]

            def gather_group(h, j, gslot):
                cnt = CNT[h][j]
                nb = nbl[h][j]
                gt3 = gbuf[gslot][:, :nb * DL[0]].rearrange(
                    "p (n d) -> p n d", d=DL[0])
                g = nc.gpsimd.dma_gather(
                    gt3,
                    agout[0][h].ap(),
                    idx_sb[:, ioff[h][j]:ioff[h][j] + cnt // 16],
                    cnt, cnt, DL[0],
                    single_packet=False,
                    queue_num=gq[0] % 4,
                )
                gq[0] += 1
                add_dep_helper(g.ins, ag_insts[0][h].ins, reason="g ag")
                st = selpool.tile([T, maxnb * T], fp8, tag="sel")
                nc.sync.dma_start(
                    out=st[:, :nb * T],
                    in_=sel[:, boff[h][j] * T:(boff[h][j] + nb) * T])
                return gt3, st, nb

            # ---- cold gathers / matmuls for L2/L3 ----
            def cold_gathers(l):
                for j in range(NT):
                    cnt = CNTC[l][j]
                    gt3 = cbuf[j][:, :nblc[l][j] * DPAD[l]].rearrange(
                        "p (n d) -> p n d", d=DPAD[l])
                    g = nc.gpsimd.dma_gather(
                        gt3,
                        agout[l][2].ap(),
                        idxc_sb[l][:, ioffc[l][j]:ioffc[l][j] + cnt // 16],
                        cnt, cnt, DPAD[l],
                        single_packet=False,
                        queue_num=gq[0] % 4,
                    )
                    gq[0] += 1
                    add_dep_helper(g.ins, ag_insts[l][2].ins, reason="cg ag")

            def cold_matmuls_chunk(l, c):
                """Fold cold edges of dst tiles 4c..4c+3 into fm[c]; the last
                one closes the accumulation group."""
                d_el = DL[l]
                for j in range(4 * c, 4 * c + 4):
                    nb = nblc[l][j]
                    gt3 = cbuf[j][:, :nb * DPAD[l]].rearrange(
                        "p (n d) -> p n d", d=DPAD[l])
                    st = selpool.tile([T, maxnb * T], fp8, tag="sel")
                    nc.sync.dma_start(
                        out=st[:, :nb * T],
                        in_=selc[l][:, boffc[l][j] * T:
                                   (boffc[l][j] + nb) * T])
                    r = (j % 4) * T
                    for b in range(nb):
                        nc.tensor.matmul(
                            fm[c][:d_el, r:r + T],
                            gt3[:, b, :d_el],
                            st[:, b * T:(b + 1) * T],
                            start=False,
                            stop=(j % 4 == 3 and b == nb - 1),
                            skip_group_check=True)

            # ---- hot scatter: chunk-a tiles then per-core b1 stripes; S
            # fetched two tiles per DMA on the Activation HWDGE queue. ----
            fetch_plan = {}           # per layer: (smat_row0, ntiles)
            hot_steps = {}
            for l in (1, 2):
                fp_ = []
                for g2 in range(SA_TILES // 2):
                    fp_.append((2 * g2, 2))
                for core in range(N_CORES):
                    base = SA_TILES + core * HOTB
                    k = 0
                    while k < B1N[l]:
                        n = min(2, B1N[l] - k)
                        fp_.append((base + k, n))
                        k += n
                fetch_plan[l] = fp_
                hs = []
                for fi, (r0, n) in enumerate(fp_):
                    for k in range(n):
                        hs.append((fi, k))
                hot_steps[l] = hs
            n_hot = {l: len(hot_steps[l]) for l in (1, 2)}

            scat = {"pos": 0, "zsb": None, "stile": None}

            def scatter_steps(l, n, limit):
                d_el = DL[l]
                dp = DPAD[l]
                b1n = B1N[l]
                while n > 0 and scat["pos"] < limit:
                    pos = scat["pos"]
                    fi, k = hot_steps[l][pos]
                    r0, fn = fetch_plan[l][fi]
                    if pos < SA_TILES:
                        if pos % 8 == 0:
                            zsb = zspool.tile([T, 8 * dp], bf16,
                                              tag=f"zsa{l}")
                            d = nc.sync.dma_start(
                                out=zsb.rearrange("p (n d) -> p n d", d=dp),
                                in_=agout_v[l][0][:, pos:pos + 8, :])
                            add_dep_helper(d.ins, ag_insts[l][0].ins,
                                           reason="zs ag")
                            scat["zsb"] = zsb
                        zk = pos % 8
                    else:
                        p = pos - SA_TILES
                        if p % b1n == 0:
                            core = p // b1n
                            zsb = zspool.tile([T, b1n * dp], bf16,
                                              tag=f"zsb{l}")
                            d = nc.sync.dma_start(
                                out=zsb.rearrange("p (n d) -> p n d", d=dp),
                                in_=agout_v[l][1][:, core * b1n:
                                                  (core + 1) * b1n, :])
                            add_dep_helper(d.ins, ag_insts[l][1].ins,
                                           reason="zs ag")
                            scat["zsb"] = zsb
                        zk = p % b1n
                    if k == 0:
                        stile = sbpool.tile([T, 2 * SHARD], fp8, tag="sm")
                        nc.scalar.dma_start(
                            out=stile[:, :fn * SHARD].rearrange(
                                "p (n d) -> p n d", d=SHARD),
                            in_=smat_v[:, r0:r0 + fn, :])
                        scat["stile"] = stile
                    stile = scat["stile"]
                    zsb = scat["zsb"]
                    for c in range(NCH):
                        nc.tensor.matmul(
                            fm[c][:d_el, :],
                            zsb[:, zk * dp:zk * dp + d_el],
                            stile[:, k * SHARD + c * CCHUNK:
                                  k * SHARD + (c + 1) * CCHUNK],
                            start=(pos == 0), stop=False,
                            skip_group_check=True)
                    scat["pos"] = pos + 1
                    n -= 1

            # ================= L1 phase A (src chunk a) ====================
            for j in range(NT):
                gt3, st, nb = gather_group(0, j, j % 3)
                ps = ps_agg.tile([T, DL[0]], f32, tag="agg")
                nc.tensor.matmul(ps[:, :], identb_sb[:, :], zb1[j][:, :],
                                 start=True, stop=False)
                for b in range(nb):
                    nc.tensor.matmul(
                        ps[:, :], st[:, b * T:(b + 1) * T], gt3[:, b, :],
                        start=False, stop=(b == nb - 1))
                nc.scalar.activation(
                    acc[:, j * DL[0]:(j + 1) * DL[0]], ps[:, :], fcopy)

            # ================= L1 phase B + post + Z2 + AG2 ================
            for j in range(NT):
                gt3, st, nb = gather_group(1, j, j % 3)
                ps = ps_agg.tile([T, DL[0]], f32, tag="agg")
                for b in range(nb):
                    nc.tensor.matmul(
                        ps[:, :], st[:, b * T:(b + 1) * T], gt3[:, b, :],
                        start=(b == 0), stop=(b == nb - 1))
                u = tpool.tile([T, DL[0]], f32, tag="post")
                nc.vector.tensor_tensor(
                    u[:, :], ps[:, :], acc[:, j * DL[0]:(j + 1) * DL[0]], add)
                h1 = hpool.tile([T, DL[0]], bf16, tag="h1")
                if apply_b1:
                    # generic path: h1 = relu(dis*u + b1); zb2 prescaled later
                    t1 = tpool.tile([T, DL[0]], f32, tag="post")
                    nc.vector.tensor_scalar(
                        t1[:, :], u[:, :], dis_sb[:, j:j + 1], None, mult)
                    t2 = tpool.tile([T, DL[0]], f32, tag="post")
                    nc.vector.tensor_tensor(t2[:, :], t1[:, :],
                                            brep1_sb[:, :], add)
                    nc.scalar.activation(h1[:, :], t2[:, :], relu)
                else:
                    # h1 = dis*relu(dis*u) = relu(dis^2*u): Z2 comes out
                    # pre-scaled for the next aggregation
                    nc.scalar.activation(h1[:, :], u[:, :], relu,
                                         scale=dis2_sb[:, j:j + 1])
                zp2 = ps_z.tile([T, 2 * DL[0]], f32, tag="zp")
                for c in range(2):
                    tp = ps_t.tile([T, T], bf16, tag="tp")
                    nc.tensor.matmul(tp[:, :], h1[:, c * T:(c + 1) * T],
                                     identb_sb[:, :], is_transpose=True)
                    htc = htpool.tile([T, T], bf16, tag="ht")
                    nc.scalar.activation(htc[:, :], tp[:, :], fcopy)
                    nc.tensor.matmul(
                        zp2[:, :DL[1]], htc[:, :],
                        w2_sb[:, c * DL[1]:(c + 1) * DL[1]],
                        start=(c == 0), stop=(c == 1))
                zb2 = zbpool.tile([T, DL[1]], bf16, tag="zb2")
                if apply_b1:
                    nc.vector.tensor_scalar(
                        zb2[:, :], zp2[:, :DL[1]], dis_sb[:, j:j + 1],
                        None, mult)
                else:
                    nc.scalar.activation(zb2[:, :], zp2[:, :DL[1]], fcopy)
                z_store(1, j, zb2)
                if j == HT_A - 1:
                    issue_ag(1, 0)
                if j == JCUT - 1:
                    issue_ag(1, 1)
                if j == NT - 1:
                    issue_ag(1, 2)
                if j >= HT_A:
                    lim = SA_TILES if j < JCUT else n_hot[1]
                    scatter_steps(1, 3, lim)

            # ---- layer tails: finish aggregation, pipeline per chunk ----
            def layer_tail(l):
                last = l == 2
                cold_gathers(l)
                scatter_steps(l, n_hot[l], n_hot[l])
                for c in range(NCH):
                    cold_matmuls_chunk(l, c)
                    d_el = DL[l]
                    t = tpool.tile([T, CCHUNK], f32, tag="fmpost")
                    if not last:
                        drow = disrow_sb if apply_b2 else disrow2_sb
                        nc.vector.tensor_tensor(
                            t[:d_el, :], fm[c][:d_el, :],
                            drow[:d_el, c * CCHUNK:(c + 1) * CCHUNK], mult)
                        nc.scalar.activation(
                            h2fm[:, c * CCHUNK:(c + 1) * CCHUNK],
                            t[:d_el, :], relu, bias=b2_sb[:, :])
                        for j in range(4 * c, 4 * c + 4):
                            zp3 = ps_z.tile([T, 2 * DL[0]], f32, tag="zp")
                            nc.tensor.matmul(
                                zp3[:, :DL[2]], h2fm[:, j * T:(j + 1) * T],
                                w3_sb[:, :], start=True, stop=True)
                            zb3 = zbpool.tile([T, DPAD[2]], bf16, tag="zb3")
                            if apply_b2:
                                nc.vector.tensor_scalar(
                                    zb3[:, :DL[2]], zp3[:, :DL[2]],
                                    dis_sb[:, j:j + 1], None, mult)
                            else:
                                nc.scalar.activation(
                                    zb3[:, :DL[2]], zp3[:, :DL[2]], fcopy)
                            z_store(2, j, zb3)
                            if j == HT_A - 1:
                                issue_ag(2, 0)
                            if j == JCUT3 - 1:
                                issue_ag(2, 1)
                            if j == NT - 1:
                                issue_ag(2, 2)
                    else:
                        nc.vector.tensor_tensor(
                            t[:d_el, :], fm[c][:d_el, :],
                            disrow_sb[:d_el, c * CCHUNK:(c + 1) * CCHUNK],
                            mult)
                        if apply_b3:
                            nc.scalar.activation(
                                outfm[:DL[2], c * CCHUNK:(c + 1) * CCHUNK],
                                t[:DL[2], :], fcopy, bias=b3_sb[:DL[2], :])
                        else:
                            nc.scalar.activation(
                                outfm[:DL[2], c * CCHUNK:(c + 1) * CCHUNK],
                                t[:DL[2], :], fcopy)
                        for j in range(4 * c, 4 * c + 4):
                            tpf = ps_z.tile([T, 2 * DL[0]], f32, tag="zp")
                            nc.tensor.matmul(
                                tpf[:, :DL[2]],
                                outfm[:DL[2], j * T:(j + 1) * T],
                                identf_sb[:DL[2], :DL[2]],
                                is_transpose=True)
                            ot = htpool.tile([T, DL[2]], f32, tag="ot")
                            nc.scalar.activation(ot[:, :], tpf[:, :DL[2]],
                                                 fcopy)
                            nc.sync.dma_start(out=out_v[:, j, :],
                                              in_=ot[:, :])

            layer_tail(1)
            scat["pos"] = 0
            layer_tail(2)

    nc.compile()
    return nc


# ----------------------------------------------------------------------------
# Host-side preprocessing
# ----------------------------------------------------------------------------
def _band_node_order(outdeg, indeg):
    by_out = np.argsort(-outdeg, kind="stable")
    node_order = -np.ones(NTAB, np.int64)
    new_pos = np.zeros(N_NODES, np.int64)
    band_sz = N_CORES * T
    for k in range(NT):
        band = by_out[k * band_sz:(k + 1) * band_sz]
        band = band[np.argsort(-indeg[band], kind="stable")]
        fill = np.zeros(N_CORES, np.int64)
        b = 0
        direction = 1
        for node in band:
            pos = b * SHARD + k * T + fill[b]
            node_order[pos] = node
            new_pos[node] = pos
            fill[b] += 1
            b += direction
            if b == N_CORES:
                b = N_CORES - 1
                direction = -1
            elif b < 0:
                b = 0
                direction = 1
    return node_order, new_pos


def _group_pack(core_s, grp, ngrp, row_s, slot_s, CNT_flat, ioff_flat,
                boff_flat, mult_s=None):
    grp_start = np.zeros(N_CORES * ngrp + 1, np.int64)
    np.add.at(grp_start, core_s * ngrp + grp + 1, 1)
    grp_start = np.cumsum(grp_start)
    rank = np.arange(len(grp)) - grp_start[core_s * ngrp + grp]
    cnt_np = np.array(CNT_flat)
    ioff_np = np.array(ioff_flat)
    boff_np = np.array(boff_flat)
    epos = ioff_np[grp] * 16 + rank
    blk = boff_np[grp] + rank // T
    lane = rank % T
    idxcols = int(ioff_np[-1] + cnt_np[-1] // 16)
    totblk = int(boff_np[-1] + (cnt_np[-1] + T - 1) // T)
    idx_cores, sel_cores = [], []
    for c in range(N_CORES):
        m = core_s == c
        flat = np.zeros(idxcols * 16, np.int16)
        flat[epos[m]] = row_s[m].astype(np.int16)
        wrapped = np.tile(flat.reshape(idxcols, 16).T, (8, 1))
        idx_cores.append(np.ascontiguousarray(wrapped.astype(np.int16)))
        selc = np.zeros((totblk, T, T), np.uint8)
        selc[blk[m], lane[m], slot_s[m]] = (
            1 if mult_s is None else mult_s[m])
        sel_cores.append(np.ascontiguousarray(
            selc.transpose(1, 0, 2).reshape(T, totblk * T)).astype(FP8))
    return idx_cores, sel_cores


def _preprocess(edge_index):
    src = np.asarray(edge_index[0], dtype=np.int64)
    dst = np.asarray(edge_index[1], dtype=np.int64)
    indeg = np.bincount(dst, minlength=N_NODES).astype(np.float64) + 1.0
    outdeg = np.bincount(src, minlength=N_NODES).astype(np.float64)
    dis_full = 1.0 / np.sqrt(indeg)

    node_order, new_pos = _band_node_order(outdeg, indeg)

    spos = new_pos[src]
    dpos = new_pos[dst]
    core = dpos // SHARD
    tile = (dpos % SHARD) // T
    slot = dpos % T
    shalf = ((spos % SHARD) >= HALF_A).astype(np.int64)
    srow_half = ((spos // SHARD) * np.where(shalf == 0, HALF_A, HALF_B)
                 + (spos % SHARD) - shalf * HALF_A)

    key = ((((core * 2 + shalf) * NT + tile) * 16384
            + srow_half) * T + slot)
    uniq, uidx, ucnt = np.unique(key, return_index=True, return_counts=True)
    core_u = core[uidx]
    shalf_u = shalf[uidx]
    tile_u = tile[uidx]
    slot_u = slot[uidx]
    srow_u = srow_half[uidx]
    counts = np.zeros((N_CORES, 2, NT), np.int64)
    np.add.at(counts, (core_u, shalf_u, tile_u), 1)
    CNT = [[max(16, _ru16(counts[:, h, j].max())) for j in range(NT)]
           for h in range(2)]
    CNT_flat = [c for row in CNT for c in row]
    _, boff_f, ioff_f, _, _ = _offsets(CNT)

    order = np.lexsort((slot_u, tile_u, shalf_u, core_u))
    g1 = shalf_u[order] * NT + tile_u[order]
    idx_cores, sel_cores = _group_pack(
        core_u[order], g1, 2 * NT, srow_u[order], slot_u[order],
        CNT_flat, ioff_f, boff_f, mult_s=ucnt[order])

    # ---- per-layer cold edges (src tile >= JC[l] -> AG chunk 2) ----
    loop_pos = new_pos[node_order[node_order >= 0]]
    s_all = np.concatenate([spos, loop_pos])
    d_all = np.concatenate([dpos, loop_pos])
    stile_all = (s_all % SHARD) // T
    idxc_cores, selc_cores, CNTC = {}, {}, {}
    for l in (1, 2):
        jc = JC[l]
        cold_m = stile_all >= jc
        sc = s_all[cold_m]
        dc = d_all[cold_m]
        ccore = dc // SHARD
        ctile = (dc % SHARD) // T
        cslot = dc % T
        crow = (sc // SHARD) * ((NT - jc) * T) + (sc % SHARD) - jc * T
        key2 = (((ccore * NT + ctile) * 32768 + crow) * T + cslot)
        uq2, ui2, uc2 = np.unique(key2, return_index=True,
                                  return_counts=True)
        ccore_u = ccore[ui2]
        ctile_u = ctile[ui2]
        cslot_u = cslot[ui2]
        crow_u = crow[ui2]
        counts2 = np.zeros((N_CORES, NT), np.int64)
        np.add.at(counts2, (ccore_u, ctile_u), 1)
        CNTC[l] = [max(16, _ru16(counts2[:, j].max())) for j in range(NT)]
        _, boff2_f, ioff2_f, _, _ = _offsets([CNTC[l]])
        order2 = np.lexsort((cslot_u, ctile_u, ccore_u))
        idxc_cores[l], selc_cores[l] = _group_pack(
            ccore_u[order2], ctile_u[order2], NT, crow_u[order2],
            cslot_u[order2], CNTC[l], ioff2_f, boff2_f, mult_s=uc2[order2])

    # ---- S matrices: hot srcs only, rows in [a | b1-hot] order ----
    hot_m = stile_all < JCUT
    sh = s_all[hot_m]
    dh = d_all[hot_m]
    sh_tile = (sh % SHARD) // T
    sh_core = sh // SHARD
    sh_off = sh % T
    in_a = sh_tile < HT_A
    srow_glob = np.where(
        in_a,
        sh_core * HALF_A + sh_tile * T + sh_off,
        N_CORES * HALF_A + sh_core * (HOTB * T)
        + (sh_tile - HT_A) * T + sh_off)
    dcore_h = dh // SHARD
    dloc_h = dh % SHARD
    smat_cores = []
    for c in range(N_CORES):
        m = dcore_h == c
        S = np.zeros((NTAB, SHARD), np.uint8)
        np.add.at(S, (srow_glob[m], dloc_h[m]), 1)
        smat_cores.append(S.astype(FP8))

    dis_cores, disrow_cores = [], []
    for c in range(N_CORES):
        slots = node_order[c * SHARD:(c + 1) * SHARD]
        dis_c = np.where(slots >= 0, dis_full[np.maximum(slots, 0)], 0.0)
        dis_cores.append(np.ascontiguousarray(
            dis_c.reshape(NT, T).T).astype(np.float32))
        disrow_cores.append(np.ascontiguousarray(
            np.tile(dis_c[None, :], (T, 1))).astype(np.float32))

    return (idx_cores, sel_cores, idxc_cores, selc_cores, dis_cores,
            disrow_cores, smat_cores, CNT, CNTC, node_order)


def _make_in_maps(x, W1, b1, W2, b2, W3, b3, edge_index):
    (idx_cores, sel_cores, idxc_cores, selc_cores, dis_cores, disrow_cores,
     smat_cores, CNT, CNTC, node_order) = _preprocess(edge_index)

    x = np.asarray(x, np.float32)
    w1b = np.asarray(W1, np.float32).astype(BF16)
    w2b = np.asarray(W2, np.float32).astype(BF16)
    w3b = np.asarray(W3, np.float32).astype(BF16)
    b1f = np.asarray(b1, np.float32)
    b2f = np.asarray(b2, np.float32)
    b3f = np.asarray(b3, np.float32)
    brep1 = np.tile(b1f, (T, 1))
    b2col = np.zeros((T, 1), np.float32)
    b2col[:DL[1], 0] = b2f
    b3col = np.zeros((T, 1), np.float32)
    b3col[:DL[2], 0] = b3f
    identb = np.eye(T, dtype=BF16)
    identf = np.eye(T, dtype=np.float32)
    apply_b1 = bool(np.any(b1f))
    apply_b2 = bool(np.any(b2f))
    apply_b3 = bool(np.any(b3f))

    in_maps = []
    for c in range(N_CORES):
        slots = node_order[c * SHARD:(c + 1) * SHARD]
        xs = np.where((slots >= 0)[:, None], x[np.maximum(slots, 0)], 0.0)
        xs = xs * disrow_cores[c][0][:, None]
        in_maps.append({
            "xt": np.ascontiguousarray(xs.T.astype(np.float32)).astype(BF16),
            "w1": w1b, "w2": w2b, "w3": w3b,
            "brep1": brep1, "b2col": b2col, "b3col": b3col,
            "dis": dis_cores[c], "dis2": dis_cores[c] ** 2,
            "disrow": disrow_cores[c].astype(BF16),
            "disrow2": (disrow_cores[c] ** 2).astype(BF16),
            "identb": identb, "identf": identf,
            "idx": idx_cores[c], "sel": sel_cores[c],
            "idxc1": idxc_cores[1][c], "selc1": selc_cores[1][c],
            "idxc2": idxc_cores[2][c], "selc2": selc_cores[2][c],
            "smat": smat_cores[c],
        })
    return in_maps, CNT, CNTC, node_order, apply_b1, apply_b2, apply_b3


_NC_CACHE = {}


def kernel_with_results(x, W1, b1, W2, b2, W3, b3, edge_index, trace=False):
    (in_maps, CNT, CNTC, node_order, apply_b1, apply_b2,
     apply_b3) = _make_in_maps(x, W1, b1, W2, b2, W3, b3, edge_index)
    key = (tuple(CNT[0]), tuple(CNT[1]), tuple(CNTC[1]), tuple(CNTC[2]),
           apply_b1, apply_b2, apply_b3)
    if key not in _NC_CACHE:
        _NC_CACHE[key] = _build_nc(CNT, CNTC, apply_b1, apply_b2, apply_b3)
    nc = _NC_CACHE[key]
    res = run_bass_kernel_spmd(
        nc, in_maps, core_ids=list(range(N_CORES)), trace=trace)
    rows = np.concatenate(
        [np.asarray(res.results[c]["out"]) for c in range(N_CORES)], axis=0)
    full = np.zeros((N_NODES, rows.shape[1]), np.float32)
    real = node_order >= 0
    full[node_order[real]] = rows[real]
    return full, res


def kernel(x, W1, b1, W2, b2, W3, b3, edge_index):
    full, _ = kernel_with_results(x, W1, b1, W2, b2, W3, b3, edge_index)
    return full
